# revision 1
# baseline (speedup 1.0000x reference)
"""Trainium2 Bass kernel for the 2-layer S4D block (nn_MetaS4History).

Strategy (8 cores, single launch):
  - Conv phases H-sharded (64 channels/core, full batch): chunked-SSD convolution
    with T=128 chunks: per-h matmuls (G-build, intra, injection) + a 16-step
    DVE scan for inter-chunk states.
  - GLU projections B-sharded (2 batch elems/core, full H): big shared-weight
    matmuls.
  - Phase boundaries resharded with AllToAll collectives (3 total).
All compute in fp32 on device; host does only layout transforms.
"""
import contextlib
import numpy as np
import concourse.bacc as bacc
import concourse.mybir as mybir
from concourse.tile import TileContext
from concourse.bass_utils import run_bass_kernel_spmd

F32 = mybir.dt.float32
AF = mybir.ActivationFunctionType
OP = mybir.AluOpType

CORES = 8
B, L, H, N = 16, 2048, 512, 64
T, C = 128, 16          # chunk len, chunk count
HS = H // CORES         # 64 channels per core
B2 = B // CORES         # 2 batch per core (GLU phase)
NHB = HS // 2           # 32 h-blocks (h = 2*hblk + hpar)
RG = [list(range(CORES))]

_NC_CACHE = {}


def _build_table(eng, tre, tim, seed_re, seed_im, mul_re, mul_im, wk, nhb):
    """Power table via doubling: tab[:, hb, j] = seed * mul^j, j in 0..T-1.
    tre/tim: [128, nhb*T] tiles; seed/mul: [128, nhb] APs (sliced); wk: pool."""
    t3re = tre[:].rearrange("p (h j) -> p h j", j=T)
    t3im = tim[:].rearrange("p (h j) -> p h j", j=T)
    eng.tensor_copy(t3re[:, :, 0:1], seed_re.unsqueeze(2))
    eng.tensor_copy(t3im[:, :, 0:1], seed_im.unsqueeze(2))
    mre = wk.tile([128, nhb], F32, tag="dbl_mre", name="dbl_mre")
    mim = wk.tile([128, nhb], F32, tag="dbl_mim", name="dbl_mim")
    q1 = wk.tile([128, nhb], F32, tag="dbl_q1", name="dbl_q1")
    q2 = wk.tile([128, nhb], F32, tag="dbl_q2", name="dbl_q2")
    sc1 = wk.tile([128, nhb * T // 2], F32, tag="dbl_s1", name="dbl_s1")
    eng.tensor_copy(mre[:], mul_re)
    eng.tensor_copy(mim[:], mul_im)
    m = 1
    while m < T:
        mbre = mre[:].unsqueeze(2).broadcast_to([128, nhb, m])
        mbim = mim[:].unsqueeze(2).broadcast_to([128, nhb, m])
        s1 = sc1[:].rearrange("p (h j) -> p h j", j=T // 2)[:, :, 0:m]
        src_re, src_im = t3re[:, :, 0:m], t3im[:, :, 0:m]
        dst_re, dst_im = t3re[:, :, m : 2 * m], t3im[:, :, m : 2 * m]
        eng.tensor_mul(s1, src_im, mbim)
        eng.tensor_mul(dst_re, src_re, mbre)
        eng.tensor_sub(dst_re, dst_re, s1)
        eng.tensor_mul(s1, src_im, mbre)
        eng.tensor_mul(dst_im, src_re, mbim)
        eng.tensor_add(dst_im, dst_im, s1)
        m *= 2
        if m < T:
            eng.tensor_mul(q1[:], mre[:], mre[:])
            eng.tensor_mul(q2[:], mim[:], mim[:])
            eng.tensor_mul(mim[:], mre[:], mim[:])
            eng.tensor_add(mim[:], mim[:], mim[:])
            eng.tensor_sub(mre[:], q1[:], q2[:])


def build_kernel(debug=False):
    key = debug
    if key in _NC_CACHE:
        return _NC_CACHE[key]
    nc = bacc.Bacc(num_devices=CORES)
    v = nc.vector
    gp = nc.gpsimd
    sc = nc.scalar
    te = nc.tensor

    # ---------------- DRAM I/O ----------------
    u0_in = nc.dram_tensor("u0", [T, B, C, HS], F32, kind="ExternalInput")
    u0b_in = nc.dram_tensor("u0b", [H, B2, L], F32, kind="ExternalInput")
    trimask_in = nc.dram_tensor("trimask", [T, T], F32, kind="ExternalInput")
    ident_in = nc.dram_tensor("ident", [T, T], F32, kind="ExternalInput")
    par_in = {}
    for l in (0, 1):
        for nm in ("ldt", "lare", "aim", "cre", "cim"):
            par_in[(nm, l)] = nc.dram_tensor(f"{nm}{l}", [128, NHB], F32, kind="ExternalInput")
        par_in[("drep", l)] = nc.dram_tensor(f"drep{l}", [128, HS], F32, kind="ExternalInput")
        par_in[("wt", l)] = nc.dram_tensor(f"wt{l}", [H, 2 * H], F32, kind="ExternalInput")
    brep0_in = nc.dram_tensor("brep0", [128, 8], F32, kind="ExternalInput")
    b1row_in = nc.dram_tensor("b1row", [1, 2 * H], F32, kind="ExternalInput")

    a2a_y_in = [nc.dram_tensor(f"a2aY{l}_in", [CORES, HS, B2, L], F32) for l in (0, 1)]
    a2a_y_out = [nc.dram_tensor(f"a2aY{l}_out", [CORES, HS, B2, L], F32) for l in (0, 1)]
    a2a_u_in = nc.dram_tensor("a2aU_in", [CORES, T, B2, C, HS], F32)
    a2a_u_out = nc.dram_tensor("a2aU_out", [CORES, T, B2, C, HS], F32)
    out_z = nc.dram_tensor("out", [B2, L, H], F32, kind="ExternalOutput")
    dbg = {}
    if debug:
        dbg["yact0"] = nc.dram_tensor("dbg_yact0", [CORES, HS, B2, L], F32, kind="ExternalOutput")
        dbg["u1"] = nc.dram_tensor("dbg_u1", [T, B, C, HS], F32, kind="ExternalOutput")

    with TileContext(nc) as tc, contextlib.ExitStack() as top:
        cpool = top.enter_context(tc.tile_pool(name="consts", bufs=1))
        trimask = cpool.tile([T, T], F32, tag="trimask", name="trimask")
        ident = cpool.tile([T, T], F32, tag="ident", name="ident")
        nc.sync.dma_start(trimask[:], trimask_in[:])
        nc.sync.dma_start(ident[:], ident_in[:])
        csts = cpool.tile([128, 32], F32, tag="csts", name="csts")
        SINC = [1.0, -1.0 / 6, 1.0 / 120, -1.0 / 5040, 1.0 / 362880, -1.0 / 39916800]
        COSC = [1.0, -1.0 / 2, 1.0 / 24, -1.0 / 720, 1.0 / 40320, -1.0 / 3628800]
        for k in range(6):
            nc.any.memset(csts[:, k : k + 1], SINC[k])
            nc.any.memset(csts[:, 6 + k : 7 + k], COSC[k])
        nc.any.memset(csts[:, 12:13], -1.0)
        nc.any.memset(csts[:, 13:14], 2.0)
        nc.any.memset(csts[:, 14:15], 1.0 / 16)
        import math
        for k in range(11):
            nc.any.memset(csts[:, 16 + k : 17 + k], 1.0 / math.factorial(k))
        nc.any.memset(csts[:, 27:28], 1.0 / 8)
        brep0 = cpool.tile([128, 8], F32, tag="brep0", name="brep0")
        nc.sync.dma_start(brep0[:], brep0_in[:])

        upool = top.enter_context(tc.tile_pool(name="u", bufs=1))
        u_sb = upool.tile([T, B * C * HS], F32, tag="u_sb", name="u_sb")  # [j,(b,c,h)]
        nc.sync.dma_start(u_sb[:], u0_in[:].rearrange("j b c h -> j (b c h)"))

        def u_slice(h, bq=None):
            b0, nb = (0, B) if bq is None else (bq * 8, 8)
            ap = u_sb[:].rearrange("j (b c h) -> j b c h", b=B, c=C)
            return ap[:, b0 : b0 + nb, :, h]

        for l in (0, 1):
            # ======== CONV PHASE (H-shard) ========
            with contextlib.ExitStack() as cv:
                pp = cv.enter_context(tc.tile_pool(name=f"par{l}", bufs=1))
                P = {}
                for nm in ("ldt", "lare", "aim", "cre", "cim"):
                    P[nm] = pp.tile([128, NHB], F32, tag=f"p_{nm}", name=f"p_{nm}")
                    nc.sync.dma_start(P[nm][:], par_in[(nm, l)][:])
                drep = pp.tile([128, HS], F32, tag="p_drep", name="p_drep")
                nc.sync.dma_start(drep[:], par_in[("drep", l)][:])

                def wk(tag):
                    return pp.tile([128, NHB], F32, tag=tag, name=tag)[:]

                neg1 = csts[:, 12:13]
                two = csts[:, 13:14]
                s16 = csts[:, 14:15]

                def exp_poly(out, x):
                    """out = e^x via (T10(x/8))^8; |x| <= 8. Accurate to ~1e-7."""
                    ea = wk("exp_a")
                    et = wk("exp_t")
                    v.tensor_scalar(ea, x, csts[:, 27:28], None, op0=OP.mult)  # y = x/8
                    v.tensor_scalar(et, ea, csts[:, 26:27], csts[:, 25:26], op0=OP.mult, op1=OP.add)
                    for k in range(8, -1, -1):
                        v.tensor_mul(et, et, ea)
                        v.tensor_scalar(et, et, csts[:, 16 + k : 17 + k], None, op0=OP.add)
                    for _ in range(3):
                        v.tensor_mul(et, et, et)
                    v.tensor_copy(out, et)

                dt, eA = wk("dt"), wk("eA")
                exp_poly(dt, P["ldt"][:])
                exp_poly(eA, P["lare"][:])
                dtAre, dtAim = wk("dtAre"), wk("dtAim")
                v.scalar_tensor_tensor(dtAre, dt, -1.0, eA, op0=OP.mult, op1=OP.mult)
                v.tensor_mul(dtAim, dt, P["aim"][:])
                mag = wk("mag")
                exp_poly(mag, dtAre)
                q, x2 = wk("q"), wk("x2")
                v.tensor_scalar(q, dtAim, s16, None, op0=OP.mult)
                v.tensor_mul(x2, q, q)
                acc, t1, t2 = wk("acc"), wk("t1"), wk("t2")
                cr, ci = wk("cr"), wk("ci")
                v.tensor_scalar(acc, x2, csts[:, 5:6], csts[:, 4:5], op0=OP.mult, op1=OP.add)
                for k in (3, 2, 1, 0):
                    v.tensor_mul(t1, acc, x2)
                    v.tensor_scalar(acc, t1, csts[:, k : k + 1], None, op0=OP.add)
                v.tensor_mul(ci, acc, q)
                v.tensor_scalar(acc, x2, csts[:, 11:12], csts[:, 10:11], op0=OP.mult, op1=OP.add)
                for k in (9, 8, 7, 6):
                    v.tensor_mul(t1, acc, x2)
                    v.tensor_scalar(acc, t1, csts[:, k : k + 1], None, op0=OP.add)
                v.tensor_copy(cr, acc)
                for _ in range(4):
                    v.tensor_mul(t1, cr, cr)
                    v.tensor_mul(t2, ci, ci)
                    v.scalar_tensor_tensor(acc, cr, 2.0, ci, op0=OP.mult, op1=OP.mult)
                    v.tensor_sub(cr, t1, t2)
                    v.tensor_copy(ci, acc)
                wre, wim = wk("wre"), wk("wim")
                v.tensor_mul(wre, mag, cr)
                v.tensor_mul(wim, mag, ci)
                m2, im2 = wk("m2"), wk("im2")
                v.tensor_mul(m2, mag, mag)
                v.reciprocal(im2, m2)
                rpre, rpim = wk("rpre"), wk("rpim")
                v.tensor_mul(rpre, wre, im2)
                v.tensor_mul(rpim, wim, im2)
                wm1re = wk("wm1re")
                v.tensor_scalar(wm1re, wre, neg1, None, op0=OP.add)
                tre, tim = wk("tre"), wk("tim")
                v.tensor_mul(t1, P["cre"][:], wm1re)
                v.tensor_mul(t2, P["cim"][:], wim)
                v.tensor_sub(tre, t1, t2)
                v.tensor_mul(t1, P["cre"][:], wim)
                v.tensor_mul(t2, P["cim"][:], wm1re)
                v.tensor_add(tim, t1, t2)
                den, invd = wk("den"), wk("invd")
                v.tensor_mul(t1, eA, eA)
                v.tensor_mul(t2, P["aim"][:], P["aim"][:])
                v.tensor_add(den, t1, t2)
                v.reciprocal(invd, den)
                ccr, cci = wk("ccr"), wk("cci")
                v.tensor_mul(t1, tre, eA)
                v.tensor_mul(t2, tim, P["aim"][:])
                v.tensor_sub(acc, t2, t1)
                v.tensor_mul(ccr, acc, invd)
                v.tensor_mul(t1, tre, P["aim"][:])
                v.tensor_mul(t2, tim, eA)
                v.tensor_add(acc, t1, t2)
                v.tensor_mul(t1, acc, invd)
                v.tensor_scalar(cci, t1, neg1, None, op0=OP.mult)
                esr, esi = wk("esr"), wk("esi")
                v.tensor_mul(t1, ccr, wre)
                v.tensor_mul(t2, cci, wim)
                v.tensor_sub(acc, t1, t2)
                v.tensor_scalar(esr, acc, two, None, op0=OP.mult)
                v.tensor_mul(t1, ccr, wim)
                v.tensor_mul(t2, cci, wre)
                v.tensor_add(acc, t1, t2)
                v.tensor_scalar(esi, acc, two, None, op0=OP.mult)
                wtr, wti = wk("wtr"), wk("wti")
                v.tensor_copy(wtr, wre)
                v.tensor_copy(wti, wim)
                for _ in range(7):
                    v.tensor_mul(t1, wtr, wtr)
                    v.tensor_mul(t2, wti, wti)
                    v.scalar_tensor_tensor(acc, wtr, 2.0, wti, op0=OP.mult, op1=OP.mult)
                    v.tensor_sub(wtr, t1, t2)
                    v.tensor_copy(wti, acc)
                dre, dim_ = wk("dre"), wk("dim")
                v.tensor_copy(dre, wtr)
                v.tensor_scalar(dim_, wti, neg1, None, op0=OP.mult)

                # ---------- conv machinery, split in hblk halves ----------
                gt_pool = cv.enter_context(tc.tile_pool(name=f"gt{l}", bufs=3))
                ya_pool = cv.enter_context(tc.tile_pool(name=f"ya{l}", bufs=2))
                HG = 8
                NHB2 = NHB // 2
                for half in (0, 1):
                  hb0 = half * NHB2
                  with contextlib.ExitStack() as hsc:
                    tpr = hsc.enter_context(tc.tile_pool(name=f"tabR{l}{half}", bufs=1))
                    Rp_re = tpr.tile([128, NHB2 * T], F32, tag="Rp_re", name="Rp_re")
                    Rp_im = tpr.tile([128, NHB2 * T], F32, tag="Rp_im", name="Rp_im")
                    with tc.tile_pool(name=f"dblR{l}{half}", bufs=1) as dwk:
                        _build_table(gp, Rp_re, Rp_im,
                                     rpre[:, hb0 : hb0 + NHB2], rpim[:, hb0 : hb0 + NHB2],
                                     rpre[:, hb0 : hb0 + NHB2], rpim[:, hb0 : hb0 + NHB2],
                                     dwk, NHB2)

                    stp = hsc.enter_context(tc.tile_pool(name=f"st{l}{half}", bufs=1))
                    X_re = stp.tile([128, NHB2 * B * C], F32, tag="X_re", name="X_re")
                    X_im = stp.tile([128, NHB2 * B * C], F32, tag="X_im", name="X_im")
                    X_re4 = X_re[:].rearrange("p (h b c) -> p h b c", b=B, c=C)
                    X_im4 = X_im[:].rearrange("p (h b c) -> p h b c", b=B, c=C)

                    # collection (transpose R' slices on the fly)
                    with tc.tile_pool(name=f"wsl{l}{half}", bufs=3) as wslp, \
                         tc.tile_pool(name=f"pst{l}{half}", bufs=2, space="PSUM") as pstp, \
                         tc.tile_pool(name=f"psc{l}{half}", bufs=2, space="PSUM") as pscp:
                        for k in range(NHB2):
                            wsl = [wslp.tile([128, T], F32, tag=f"wsl{comp}", name=f"wsl{comp}")
                                   for comp in (0, 1)]
                            for comp, Rt in enumerate((Rp_re, Rp_im)):
                                psT = pstp.tile([128, T], F32, tag="psT", name="psT")
                                te.transpose(psT[:], Rt[:, k * T : (k + 1) * T], ident[:])
                                sc.activation(wsl[comp][:], psT[:], AF.Copy)
                            psr = pscp.tile([128, B * C], F32, tag="psr", name="psr")
                            psi = pscp.tile([128, B * C], F32, tag="psi", name="psi")
                            for hp in (0, 1):
                                h = 2 * (hb0 + k) + hp
                                us = u_slice(h)
                                te.matmul(psr[64 * hp : 64 * hp + 64, :],
                                          wsl[0][:, 64 * hp : 64 * hp + 64], us, start=True, stop=True)
                                te.matmul(psi[64 * hp : 64 * hp + 64, :],
                                          wsl[1][:, 64 * hp : 64 * hp + 64], us, start=True, stop=True)
                            sc.activation(X_re4[:, k, :, :], psr[:], AF.Copy)
                            sc.activation(X_im4[:, k, :, :], psi[:], AF.Copy)

                    # scan (in place: X becomes Sacc)
                    with tc.tile_pool(name=f"scan{l}{half}", bufs=1) as sp:
                        def stile(nm):
                            return sp.tile([128, NHB2 * B], F32, tag=nm, name=nm)[:].rearrange(
                                "p (h b) -> p h b", b=B)
                        Sr3, Si3 = stile("Sr"), stile("Si")
                        t_r3, t_i3 = stile("tm_r"), stile("tm_i")
                        w13, w23 = stile("w1"), stile("w2")
                        nc.any.memset(Sr3, 0.0)
                        nc.any.memset(Si3, 0.0)
                        dreb = dre[:, hb0 : hb0 + NHB2].unsqueeze(2).broadcast_to([128, NHB2, B])
                        dimb = dim_[:, hb0 : hb0 + NHB2].unsqueeze(2).broadcast_to([128, NHB2, B])
                        for ccc in range(C):
                            xr, xi = X_re4[:, :, :, ccc], X_im4[:, :, :, ccc]
                            v.tensor_add(t_r3, Sr3, xr)
                            gp.tensor_add(t_i3, Si3, xi)
                            sc.activation(xr, Sr3, AF.Copy)
                            sc.activation(xi, Si3, AF.Copy)
                            v.tensor_mul(w13, t_r3, dreb)
                            v.tensor_mul(w23, t_i3, dimb)
                            v.tensor_sub(Sr3, w13, w23)
                            v.tensor_mul(w13, t_i3, dreb)
                            v.tensor_mul(w23, t_r3, dimb)
                            v.tensor_add(Si3, w13, w23)

                    # E table for this half
                    gp_ps = hsc.enter_context(tc.tile_pool(name=f"gps{l}{half}", bufs=2, space="PSUM"))
                    cv_ps = hsc.enter_context(tc.tile_pool(name=f"cvps{l}{half}", bufs=2, space="PSUM"))
                    tpe = hsc.enter_context(tc.tile_pool(name=f"tabE{l}{half}", bufs=1))
                    E_re = tpe.tile([128, NHB2 * T], F32, tag="E_re", name="E_re")
                    E_im = tpe.tile([128, NHB2 * T], F32, tag="E_im", name="E_im")
                    with tc.tile_pool(name=f"dblE{l}{half}", bufs=1) as dwk:
                        _build_table(v, E_re, E_im,
                                     esr[:, hb0 : hb0 + NHB2], esi[:, hb0 : hb0 + NHB2],
                                     wre[:, hb0 : hb0 + NHB2], wim[:, hb0 : hb0 + NHB2],
                                     dwk, NHB2)

                    # per-h conv
                    yg = [None, None]
                    for hh in range(HS // 2):
                        h = 2 * hb0 + hh
                        hp, hb = h & 1, h >> 1
                        base = 64 * hp
                        kb = hb - hb0
                        er = E_re[base : base + 64, kb * T : (kb + 1) * T]
                        ei = E_im[base : base + 64, kb * T : (kb + 1) * T]
                        rr = Rp_re[base : base + 64, kb * T : (kb + 1) * T]
                        ri = Rp_im[base : base + 64, kb * T : (kb + 1) * T]
                        psG = gp_ps.tile([128, T], F32, tag="psG", name="psG")
                        te.matmul(psG[:], rr, er, start=True, stop=False)
                        te.matmul(psG[:], ri, ei, start=False, stop=True)
                        GTt = gt_pool.tile([128, T], F32, tag="GTt", name="GTt")
                        GT = gt_pool.tile([128, T], F32, tag="GT", name="GT")
                        v.tensor_mul(GTt[:], psG[:], trimask[:])
                        v.scalar_tensor_tensor(GT[:], ident[:], drep[:, h : h + 1], GTt[:],
                                               op0=OP.mult, op1=OP.add)
                        if hh % HG == 0:
                            yg = [ya_pool.tile([128, HG * T], F32, tag=f"yg{qq}", name=f"yg{qq}")
                                  for qq in (0, 1)]
                        for qq in (0, 1):
                            ps = cv_ps.tile([128, T], F32, tag="ps", name="ps")
                            lu = u_slice(h, qq)
                            te.matmul(ps[:], lu, GT[:], start=True, stop=False)
                            lr = X_re[base : base + 64,
                                      kb * B * C + qq * 128 : kb * B * C + qq * 128 + 128]
                            li = X_im[base : base + 64,
                                      kb * B * C + qq * 128 : kb * B * C + qq * 128 + 128]
                            te.matmul(ps[:], lr, er, start=False, stop=False)
                            te.matmul(ps[:], li, ei, start=False, stop=True)
                            sc.activation(yg[qq][:, (hh % HG) * T : (hh % HG + 1) * T], ps[:],
                                          AF.Gelu_apprx_tanh)
                        if hh % HG == HG - 1:
                            hg0 = h - HG + 1
                            for qq in (0, 1):
                                ygv = yg[qq][:].rearrange("bc (hh2 j) -> bc hh2 j", j=T)
                                for dd in range(4):
                                    d = qq * 4 + dd
                                    dst = a2a_y_in[l][d, hg0 : hg0 + HG, :, :].rearrange(
                                        "hh2 b2 (c j) -> (b2 c) hh2 j", j=T)
                                    nc.sync.dma_start(dst, ygv[32 * dd : 32 * dd + 32, :, :])

            # ======== AllToAll y ========
            gp.collective_compute(
                "AllToAll", OP.bypass, replica_groups=RG,
                ins=[a2a_y_in[l][:].opt()], outs=[a2a_y_out[l][:].opt()])

            # ======== GLU PHASE (B-shard) ========
            with contextlib.ExitStack() as gl:
                gpool = gl.enter_context(tc.tile_pool(name=f"glu{l}", bufs=1))
                wtiles = [gpool.tile([128, 2 * H], F32, tag=f"wt{k}", name=f"wt{k}") for k in range(4)]
                ytiles = [gpool.tile([128, B2 * L], F32, tag=f"yk{k}", name=f"yk{k}") for k in range(4)]
                for kt in range(4):
                    nc.sync.dma_start(wtiles[kt][:], par_in[("wt", l)][128 * kt : 128 * (kt + 1), :])
                    src = a2a_y_out[l][:].rearrange("s h b2 ll -> (s h) (b2 ll)")
                    nc.sync.dma_start(ytiles[kt][:], src[128 * kt : 128 * (kt + 1), :])
                if debug and l == 0:
                    for s in range(CORES):
                        gb = gpool.tile([64, B2 * L], F32, tag="dbgy", name="dbgy")
                        nc.sync.dma_start(gb[:], a2a_y_out[l][s].rearrange("h b2 ll -> h (b2 ll)"))
                        nc.sync.dma_start(dbg["yact0"][s].rearrange("h b2 ll -> h (b2 ll)"), gb[:])
                zps = gl.enter_context(tc.tile_pool(name=f"zps{l}", bufs=2, space="PSUM"))
                if l == 0:
                    zwp = gl.enter_context(tc.tile_pool(name=f"zw{l}", bufs=3))
                    ubp = gl.enter_context(tc.tile_pool(name=f"ub{l}", bufs=3))
                    tps = gl.enter_context(tc.tile_pool(name=f"tps{l}", bufs=2, space="PSUM"))
                    trp = gl.enter_context(tc.tile_pool(name=f"trp{l}", bufs=3))
                    u0bf = u0b_in[:].rearrange("ch b2 ll -> ch (b2 ll)")
                    for kt in range(4):
                        for ch in range(8):
                            sl = slice(ch * 512, (ch + 1) * 512)
                            psZ = zps.tile([128, 512], F32, tag="psZ", name="psZ")
                            for k2 in range(4):
                                te.matmul(psZ[:], wtiles[k2][:, kt * 128 : (kt + 1) * 128],
                                          ytiles[k2][:, sl], start=(k2 == 0), stop=(k2 == 3))
                            z1c = zwp.tile([128, 512], F32, tag="z1c", name="z1c")
                            v.tensor_scalar(z1c[:], psZ[:], brep0[:, kt : kt + 1], None, op0=OP.add)
                            psZ2 = zps.tile([128, 512], F32, tag="psZ2", name="psZ2")
                            for k2 in range(4):
                                te.matmul(psZ2[:], wtiles[k2][:, (kt + 4) * 128 : (kt + 5) * 128],
                                          ytiles[k2][:, sl], start=(k2 == 0), stop=(k2 == 3))
                            sgc = zwp.tile([128, 512], F32, tag="sgc", name="sgc")
                            sc.activation(sgc[:], psZ2[:], AF.Sigmoid, bias=brep0[:, kt + 4 : kt + 5])
                            ub = ubp.tile([128, 512], F32, tag="ub", name="ub")
                            nc.sync.dma_start(ub[:], u0bf[128 * kt : 128 * (kt + 1), sl])
                            v.tensor_mul(z1c[:], z1c[:], sgc[:])
                            v.tensor_add(z1c[:], z1c[:], ub[:])
                            # transpose the 4 l-tiles of this chunk and send
                            b2c = ch // 4
                            for c4 in range(4):
                                ccc = (ch % 4) * 4 + c4
                                psT = tps.tile([128, 128], F32, tag="psT2", name="psT2")
                                te.transpose(psT[:], z1c[:, c4 * 128 : (c4 + 1) * 128], ident[:])
                                trsb = trp.tile([128, 128], F32, tag="trsb", name="trsb")
                                sc.activation(trsb[:], psT[:], AF.Copy)
                                dst = a2a_u_in[:, :, b2c, ccc, :][2 * kt : 2 * kt + 2].rearrange(
                                    "e j hh -> j e hh")
                                nc.sync.dma_start(dst, trsb[:].rearrange("j (e hh) -> j e hh", hh=64))
                    gp.collective_compute(
                        "AllToAll", OP.bypass, replica_groups=RG,
                        ins=[a2a_u_in[:].opt()], outs=[a2a_u_out[:].opt()])
                    for s in range(CORES):
                        src = a2a_u_out[s].rearrange("j b2 c h -> j (b2 c h)")
                        dstv = u_sb[:].rearrange("j (b c h) -> j b c h", b=B, c=C)[
                            :, 2 * s : 2 * s + 2, :, :].rearrange("j b c h -> j (b c h)")
                        nc.sync.dma_start(dstv, src)
                    if debug:
                        for jj in range(2):
                            nc.sync.dma_start(
                                dbg["u1"][64 * jj : 64 * jj + 64].rearrange("j b c h -> j (b c h)"),
                                u_sb[64 * jj : 64 * jj + 64, :])
                else:
                    zw1 = gl.enter_context(tc.tile_pool(name=f"zw1{l}", bufs=3))
                    b1b = gpool.tile([128, 2 * H], F32, tag="b1b", name="b1b")
                    nc.sync.dma_start(b1b[:], b1row_in[:].broadcast_to([128, 2 * H]))
                    for b2 in range(B2):
                        for lt in range(C):
                            zz = []
                            for oh in (0, 1):
                                psW = zps.tile([128, 512], F32, tag="psW", name="psW")
                                for kt in range(4):
                                    te.matmul(psW[:],
                                              ytiles[kt][:, b2 * L + lt * T : b2 * L + (lt + 1) * T],
                                              wtiles[kt][:, oh * 512 : (oh + 1) * 512],
                                              start=(kt == 0), stop=(kt == 3))
                                zt = zw1.tile([128, 512], F32, tag=f"zt{oh}", name=f"zt{oh}")
                                v.tensor_add(zt[:], psW[:], b1b[:, oh * 512 : (oh + 1) * 512])
                                zz.append(zt)
                            sg = zw1.tile([128, 512], F32, tag="sg1", name="sg1")
                            sc.activation(sg[:], zz[1][:], AF.Sigmoid)
                            osb = zw1.tile([128, 512], F32, tag="osb", name="osb")
                            v.tensor_mul(osb[:], zz[0][:], sg[:])
                            nc.sync.dma_start(out_z[b2, lt * T : (lt + 1) * T, :], osb[:])
    nc.finalize()
    _NC_CACHE[key] = nc
    return nc


# ====================== host side ======================

def _prep_core_inputs(core, x, pars):
    hs = slice(HS * core, HS * (core + 1))
    ins = {}
    xs = x[:, :, hs]                                    # (B, L, 64)
    u0 = xs.reshape(B, C, T, HS).transpose(2, 0, 1, 3)  # (j, b, c, h)
    ins["u0"] = np.ascontiguousarray(u0)
    xb = x[B2 * core : B2 * (core + 1)]                 # (2, L, H)
    ins["u0b"] = np.ascontiguousarray(xb.transpose(2, 0, 1))
    ins["trimask"] = np.triu(np.ones((T, T), np.float32))
    ins["ident"] = np.eye(T, dtype=np.float32)

    def scan_layout(a):
        if a.ndim == 1:
            a = np.broadcast_to(a[:, None], (HS, N))
        return np.ascontiguousarray(
            a.reshape(NHB, 2, N).transpose(1, 2, 0).reshape(128, NHB))

    for l in (0, 1):
        ins[f"ldt{l}"] = scan_layout(pars[f"ldt{l}"][hs])
        ins[f"lare{l}"] = scan_layout(pars[f"lAre{l}"][hs])
        ins[f"aim{l}"] = scan_layout(pars[f"Aim{l}"][hs])
        ins[f"cre{l}"] = scan_layout(pars[f"Cre{l}"][hs])
        ins[f"cim{l}"] = scan_layout(pars[f"Cim{l}"][hs])
        ins[f"drep{l}"] = np.ascontiguousarray(
            np.broadcast_to(pars[f"D{l}"][hs][None, :], (128, HS)))
        ins[f"wt{l}"] = np.ascontiguousarray(pars[f"W{l}"].T)
    ins["brep0"] = np.ascontiguousarray(pars["b0"].reshape(8, 128).T)
    ins["b1row"] = np.ascontiguousarray(pars["b1"][None, :])
    return {k: vv.astype(np.float32) for k, vv in ins.items()}


def run(x, pars, debug=False, trace=False):
    nc = build_kernel(debug=debug)
    in_maps = [_prep_core_inputs(c, x, pars) for c in range(CORES)]
    r = run_bass_kernel_spmd(nc, in_maps, core_ids=list(range(CORES)), trace=trace)
    outs = np.stack([r.results[c]["out"] for c in range(CORES)])  # (8, 2, L, H)
    full = outs.reshape(B, L, H)
    return full, r


def kernel(**inputs):
    x = np.asarray(inputs["x"], dtype=np.float32)
    pars = {k: np.asarray(vv, dtype=np.float32) for k, vv in inputs.items() if k != "x"}
    full, _ = run(x, pars)
    return full



# revision 19
# speedup vs baseline: 1.6093x; 1.6093x over previous
"""Trainium2 Bass kernel for the 2-layer S4D block (nn_MetaS4History).

Strategy (8 cores, single launch):
  - Conv phases H-sharded (64 channels/core, full batch): chunked-SSD convolution
    with T=128 chunks: per-h matmuls (G-build, intra, injection) + a 16-step
    DVE scan for inter-chunk states.
  - GLU projections B-sharded (2 batch elems/core, full H): big shared-weight
    matmuls.
  - Phase boundaries resharded with AllToAll collectives (3 total).
All compute in fp32 on device; host does only layout transforms.
"""
import contextlib
import ml_dtypes
import numpy as np
import concourse.bacc as bacc
import concourse.mybir as mybir
from concourse.tile import TileContext
from concourse.bass_utils import run_bass_kernel_spmd

F32 = mybir.dt.float32
F32R = mybir.dt.float32r
BF16 = mybir.dt.bfloat16
AF = mybir.ActivationFunctionType
OP = mybir.AluOpType

CORES = 8
B, L, H, N = 16, 2048, 512, 64
T, C = 128, 16          # chunk len, chunk count
HS = H // CORES         # 64 channels per core
B2 = B // CORES         # 2 batch per core (GLU phase)
NHB = HS // 2           # 32 h-blocks (h = 2*hblk + hpar)
RG = [list(range(CORES))]

_NC_CACHE = {}


def _build_table(eng, tre, tim, seed_re, seed_im, mul_re, mul_im, wk, nhb):
    """Power table via doubling: tab[:, hb, j] = seed * mul^j, j in 0..T-1.
    tre/tim: [128, nhb*T] tiles; seed/mul: [128, nhb] APs (sliced); wk: pool."""
    t3re = tre[:].rearrange("p (h j) -> p h j", j=T)
    t3im = tim[:].rearrange("p (h j) -> p h j", j=T)
    eng.tensor_copy(t3re[:, :, 0:1], seed_re.unsqueeze(2))
    eng.tensor_copy(t3im[:, :, 0:1], seed_im.unsqueeze(2))
    mre = wk.tile([128, nhb], F32, tag="dbl_mre", name="dbl_mre")
    mim = wk.tile([128, nhb], F32, tag="dbl_mim", name="dbl_mim")
    q1 = wk.tile([128, nhb], F32, tag="dbl_q1", name="dbl_q1")
    q2 = wk.tile([128, nhb], F32, tag="dbl_q2", name="dbl_q2")
    sc1 = wk.tile([128, nhb * T // 2], F32, tag="dbl_s1", name="dbl_s1")
    eng.tensor_copy(mre[:], mul_re)
    eng.tensor_copy(mim[:], mul_im)
    m = 1
    while m < T:
        mbre = mre[:].unsqueeze(2).broadcast_to([128, nhb, m])
        mbim = mim[:].unsqueeze(2).broadcast_to([128, nhb, m])
        s1 = sc1[:].rearrange("p (h j) -> p h j", j=T // 2)[:, :, 0:m]
        src_re, src_im = t3re[:, :, 0:m], t3im[:, :, 0:m]
        dst_re, dst_im = t3re[:, :, m : 2 * m], t3im[:, :, m : 2 * m]
        eng.tensor_mul(s1, src_im, mbim)
        eng.tensor_mul(dst_re, src_re, mbre)
        eng.tensor_sub(dst_re, dst_re, s1)
        eng.tensor_mul(s1, src_im, mbre)
        eng.tensor_mul(dst_im, src_re, mbim)
        eng.tensor_add(dst_im, dst_im, s1)
        m *= 2
        if m < T:
            eng.tensor_mul(q1[:], mre[:], mre[:])
            eng.tensor_mul(q2[:], mim[:], mim[:])
            eng.tensor_mul(mim[:], mre[:], mim[:])
            eng.tensor_add(mim[:], mim[:], mim[:])
            eng.tensor_sub(mre[:], q1[:], q2[:])


def build_kernel(debug=False):
    key = debug
    if key in _NC_CACHE:
        return _NC_CACHE[key]
    nc = bacc.Bacc(num_devices=CORES)
    v = nc.vector
    gp = nc.gpsimd
    sc = nc.scalar
    te = nc.tensor

    # ---------------- DRAM I/O ----------------
    u0_in = nc.dram_tensor("u0", [T, B, C, HS], BF16, kind="ExternalInput")
    u0b_in = nc.dram_tensor("u0b", [H, B2, L], F32, kind="ExternalInput")
    trimask_in = nc.dram_tensor("trimask", [T, T], F32, kind="ExternalInput")
    ident_in = nc.dram_tensor("ident", [T, T], F32, kind="ExternalInput")
    par_in = {}
    for l in (0, 1):
        for nm in ("ldt", "lare", "aim", "cre", "cim"):
            par_in[(nm, l)] = nc.dram_tensor(f"{nm}{l}", [128, NHB], F32, kind="ExternalInput")
        par_in[("drep", l)] = nc.dram_tensor(f"drep{l}", [128, HS], F32, kind="ExternalInput")
        par_in[("wt", l)] = nc.dram_tensor(f"wt{l}", [H, 2 * H], BF16, kind="ExternalInput")
    brep0_in = nc.dram_tensor("brep0", [128, 8], F32, kind="ExternalInput")
    b1row_in = nc.dram_tensor("b1row", [1, 2 * H], F32, kind="ExternalInput")

    a2a_y_in = [nc.dram_tensor(f"a2aY{l}_in", [CORES, HS, B2, L], BF16) for l in (0, 1)]
    a2a_y_out = [nc.dram_tensor(f"a2aY{l}_out", [CORES, HS, B2, L], BF16) for l in (0, 1)]
    a2a_u_in = nc.dram_tensor("a2aU_in", [CORES, T, B2, C, HS], BF16)
    a2a_u_out = nc.dram_tensor("a2aU_out", [CORES, T, B2, C, HS], BF16)
    out_z = nc.dram_tensor("out", [B2, L, H], F32, kind="ExternalOutput")
    dbg = {}
    if debug:
        dbg["yact0"] = nc.dram_tensor("dbg_yact0", [CORES, HS, B2, L], BF16, kind="ExternalOutput")
        dbg["u1"] = nc.dram_tensor("dbg_u1", [T, B, C, HS], BF16, kind="ExternalOutput")

    with TileContext(nc) as tc, contextlib.ExitStack() as top:
        cpool = top.enter_context(tc.tile_pool(name="consts", bufs=1))
        trimask = cpool.tile([T, T], F32, tag="trimask", name="trimask")
        ident = cpool.tile([T, T], F32, tag="ident", name="ident")
        nc.sync.dma_start(trimask[:], trimask_in[:])
        nc.sync.dma_start(ident[:], ident_in[:])
        csts = cpool.tile([128, 32], F32, tag="csts", name="csts")
        SINC = [1.0, -1.0 / 6, 1.0 / 120, -1.0 / 5040, 1.0 / 362880, -1.0 / 39916800]
        COSC = [1.0, -1.0 / 2, 1.0 / 24, -1.0 / 720, 1.0 / 40320, -1.0 / 3628800]
        for k in range(6):
            nc.any.memset(csts[:, k : k + 1], SINC[k])
            nc.any.memset(csts[:, 6 + k : 7 + k], COSC[k])
        nc.any.memset(csts[:, 12:13], -1.0)
        nc.any.memset(csts[:, 13:14], 2.0)
        nc.any.memset(csts[:, 14:15], 1.0 / 16)
        import math
        for k in range(11):
            nc.any.memset(csts[:, 16 + k : 17 + k], 1.0 / math.factorial(k))
        nc.any.memset(csts[:, 27:28], 1.0 / 8)
        brep0 = cpool.tile([128, 8], F32, tag="brep0", name="brep0")
        nc.sync.dma_start(brep0[:], brep0_in[:])

        upool = top.enter_context(tc.tile_pool(name="u", bufs=1))
        u_sb = upool.tile([T, B * C * HS], BF16, tag="u_sb", name="u_sb")  # [j,(b,c,h)]
        nc.sync.dma_start(u_sb[:], u0_in[:].rearrange("j b c h -> j (b c h)"))

        def u_slice(h, bq=None):
            b0, nb = (0, B) if bq is None else (bq * 8, 8)
            ap = u_sb[:].rearrange("j (b c h) -> j b c h", b=B, c=C)
            return ap[:, b0 : b0 + nb, :, h]

        for l in (0, 1):
            # ======== CONV PHASE (H-shard) ========
            with contextlib.ExitStack() as cv:
                pp = cv.enter_context(tc.tile_pool(name=f"par{l}", bufs=1))
                P = {}
                for nm in ("ldt", "lare", "aim", "cre", "cim"):
                    P[nm] = pp.tile([128, NHB], F32, tag=f"p_{nm}", name=f"p_{nm}")
                    nc.sync.dma_start(P[nm][:], par_in[(nm, l)][:])
                drep = pp.tile([128, HS], F32, tag="p_drep", name="p_drep")
                nc.sync.dma_start(drep[:], par_in[("drep", l)][:])

                def wk(tag):
                    return pp.tile([128, NHB], F32, tag=tag, name=tag)[:]

                neg1 = csts[:, 12:13]
                two = csts[:, 13:14]
                s16 = csts[:, 14:15]

                def exp_poly(out, x):
                    """out = e^x via (T10(x/8))^8; |x| <= 8. Accurate to ~1e-7."""
                    ea = wk("exp_a")
                    et = wk("exp_t")
                    v.tensor_scalar(ea, x, csts[:, 27:28], None, op0=OP.mult)  # y = x/8
                    v.tensor_scalar(et, ea, csts[:, 26:27], csts[:, 25:26], op0=OP.mult, op1=OP.add)
                    for k in range(8, -1, -1):
                        v.tensor_mul(et, et, ea)
                        v.tensor_scalar(et, et, csts[:, 16 + k : 17 + k], None, op0=OP.add)
                    for _ in range(3):
                        v.tensor_mul(et, et, et)
                    v.tensor_copy(out, et)

                dt, eA = wk("dt"), wk("eA")
                exp_poly(dt, P["ldt"][:])
                exp_poly(eA, P["lare"][:])
                dtAre, dtAim = wk("dtAre"), wk("dtAim")
                v.scalar_tensor_tensor(dtAre, dt, -1.0, eA, op0=OP.mult, op1=OP.mult)
                v.tensor_mul(dtAim, dt, P["aim"][:])
                mag = wk("mag")
                exp_poly(mag, dtAre)
                q, x2 = wk("q"), wk("x2")
                v.tensor_scalar(q, dtAim, s16, None, op0=OP.mult)
                v.tensor_mul(x2, q, q)
                acc, t1, t2 = wk("acc"), wk("t1"), wk("t2")
                cr, ci = wk("cr"), wk("ci")
                v.tensor_scalar(acc, x2, csts[:, 5:6], csts[:, 4:5], op0=OP.mult, op1=OP.add)
                for k in (3, 2, 1, 0):
                    v.tensor_mul(t1, acc, x2)
                    v.tensor_scalar(acc, t1, csts[:, k : k + 1], None, op0=OP.add)
                v.tensor_mul(ci, acc, q)
                v.tensor_scalar(acc, x2, csts[:, 11:12], csts[:, 10:11], op0=OP.mult, op1=OP.add)
                for k in (9, 8, 7, 6):
                    v.tensor_mul(t1, acc, x2)
                    v.tensor_scalar(acc, t1, csts[:, k : k + 1], None, op0=OP.add)
                v.tensor_copy(cr, acc)
                for _ in range(4):
                    v.tensor_mul(t1, cr, cr)
                    v.tensor_mul(t2, ci, ci)
                    v.scalar_tensor_tensor(acc, cr, 2.0, ci, op0=OP.mult, op1=OP.mult)
                    v.tensor_sub(cr, t1, t2)
                    v.tensor_copy(ci, acc)
                wre, wim = wk("wre"), wk("wim")
                v.tensor_mul(wre, mag, cr)
                v.tensor_mul(wim, mag, ci)
                m2, im2 = wk("m2"), wk("im2")
                v.tensor_mul(m2, mag, mag)
                v.reciprocal(im2, m2)
                rpre, rpim = wk("rpre"), wk("rpim")
                v.tensor_mul(rpre, wre, im2)
                v.tensor_mul(rpim, wim, im2)
                wm1re = wk("wm1re")
                v.tensor_scalar(wm1re, wre, neg1, None, op0=OP.add)
                tre, tim = wk("tre"), wk("tim")
                v.tensor_mul(t1, P["cre"][:], wm1re)
                v.tensor_mul(t2, P["cim"][:], wim)
                v.tensor_sub(tre, t1, t2)
                v.tensor_mul(t1, P["cre"][:], wim)
                v.tensor_mul(t2, P["cim"][:], wm1re)
                v.tensor_add(tim, t1, t2)
                den, invd = wk("den"), wk("invd")
                v.tensor_mul(t1, eA, eA)
                v.tensor_mul(t2, P["aim"][:], P["aim"][:])
                v.tensor_add(den, t1, t2)
                v.reciprocal(invd, den)
                ccr, cci = wk("ccr"), wk("cci")
                v.tensor_mul(t1, tre, eA)
                v.tensor_mul(t2, tim, P["aim"][:])
                v.tensor_sub(acc, t2, t1)
                v.tensor_mul(ccr, acc, invd)
                v.tensor_mul(t1, tre, P["aim"][:])
                v.tensor_mul(t2, tim, eA)
                v.tensor_add(acc, t1, t2)
                v.tensor_mul(t1, acc, invd)
                v.tensor_scalar(cci, t1, neg1, None, op0=OP.mult)
                esr, esi = wk("esr"), wk("esi")
                v.tensor_mul(t1, ccr, wre)
                v.tensor_mul(t2, cci, wim)
                v.tensor_sub(acc, t1, t2)
                v.tensor_scalar(esr, acc, two, None, op0=OP.mult)
                v.tensor_mul(t1, ccr, wim)
                v.tensor_mul(t2, cci, wre)
                v.tensor_add(acc, t1, t2)
                v.tensor_scalar(esi, acc, two, None, op0=OP.mult)
                wtr, wti = wk("wtr"), wk("wti")
                v.tensor_copy(wtr, wre)
                v.tensor_copy(wti, wim)
                for _ in range(7):
                    v.tensor_mul(t1, wtr, wtr)
                    v.tensor_mul(t2, wti, wti)
                    v.scalar_tensor_tensor(acc, wtr, 2.0, wti, op0=OP.mult, op1=OP.mult)
                    v.tensor_sub(wtr, t1, t2)
                    v.tensor_copy(wti, acc)
                dre, dim_ = wk("dre"), wk("dim")
                v.tensor_copy(dre, wtr)
                v.tensor_scalar(dim_, wti, neg1, None, op0=OP.mult)

                # ---------- conv machinery, split in hblk halves ----------
                gt_pool = cv.enter_context(tc.tile_pool(name=f"gt{l}", bufs=3))
                ya_pool = cv.enter_context(tc.tile_pool(name=f"ya{l}", bufs=2))
                HG = 8
                NHB2 = NHB // 2
                for half in (0, 1):
                  hb0 = half * NHB2
                  with contextlib.ExitStack() as hsc:
                    tpr = hsc.enter_context(tc.tile_pool(name=f"tabR{l}{half}", bufs=1))
                    Rp_re = tpr.tile([128, NHB2 * T], F32, tag="Rp_re", name="Rp_re")
                    Rp_im = tpr.tile([128, NHB2 * T], F32, tag="Rp_im", name="Rp_im")
                    R16re = tpr.tile([128, NHB2 * T], BF16, tag="R16re", name="R16re")
                    R16im = tpr.tile([128, NHB2 * T], BF16, tag="R16im", name="R16im")
                    with tc.tile_pool(name=f"dblR{l}{half}", bufs=1) as dwk:
                        _build_table(gp, Rp_re, Rp_im,
                                     rpre[:, hb0 : hb0 + NHB2], rpim[:, hb0 : hb0 + NHB2],
                                     rpre[:, hb0 : hb0 + NHB2], rpim[:, hb0 : hb0 + NHB2],
                                     dwk, NHB2)
                    gp.tensor_copy(R16re[:], Rp_re[:])
                    gp.tensor_copy(R16im[:], Rp_im[:])

                    stp = hsc.enter_context(tc.tile_pool(name=f"st{l}{half}", bufs=1))
                    X_re = stp.tile([128, NHB2 * B * C], BF16, tag="X_re", name="X_re")
                    X_im = stp.tile([128, NHB2 * B * C], BF16, tag="X_im", name="X_im")
                    X_re4 = X_re[:].rearrange("p (h b c) -> p h b c", b=B, c=C)
                    X_im4 = X_im[:].rearrange("p (h b c) -> p h b c", b=B, c=C)

                    # collection (transpose R' slices on the fly)
                    with tc.tile_pool(name=f"wsl{l}{half}", bufs=3) as wslp, \
                         tc.tile_pool(name=f"pst{l}{half}", bufs=2, space="PSUM") as pstp, \
                         tc.tile_pool(name=f"psc{l}{half}", bufs=2, space="PSUM") as pscp:
                        for k in range(NHB2):
                            wsl = [wslp.tile([128, T], BF16, tag=f"wsl{comp}", name=f"wsl{comp}")
                                   for comp in (0, 1)]
                            for comp, Rt in enumerate((Rp_re, Rp_im)):
                                psT = pstp.tile([128, T], F32, tag="psT", name="psT")
                                te.transpose(psT[:], Rt[:, k * T : (k + 1) * T], ident[:])
                                sc.activation(wsl[comp][:], psT[:], AF.Copy)
                            psr = pscp.tile([128, B * C], F32, tag="psr", name="psr")
                            psi = pscp.tile([128, B * C], F32, tag="psi", name="psi")
                            for hp in (0, 1):
                                h = 2 * (hb0 + k) + hp
                                us = u_slice(h)
                                te.matmul(psr[64 * hp : 64 * hp + 64, :],
                                          wsl[0][:, 64 * hp : 64 * hp + 64], us, start=True, stop=True)
                                te.matmul(psi[64 * hp : 64 * hp + 64, :],
                                          wsl[1][:, 64 * hp : 64 * hp + 64], us, start=True, stop=True)
                            sc.activation(X_re4[:, k, :, :], psr[:], AF.Copy)
                            sc.activation(X_im4[:, k, :, :], psi[:], AF.Copy)

                    # scan (in place: X becomes Sacc)
                    with tc.tile_pool(name=f"scan{l}{half}", bufs=1) as sp:
                        def stile(nm):
                            return sp.tile([128, NHB2 * B], F32, tag=nm, name=nm)[:].rearrange(
                                "p (h b) -> p h b", b=B)
                        Sr3, Si3 = stile("Sr"), stile("Si")
                        t_r3, t_i3 = stile("tm_r"), stile("tm_i")
                        w13, w23 = stile("w1"), stile("w2")
                        nc.any.memset(Sr3, 0.0)
                        nc.any.memset(Si3, 0.0)
                        dreb = dre[:, hb0 : hb0 + NHB2].unsqueeze(2).broadcast_to([128, NHB2, B])
                        dimb = dim_[:, hb0 : hb0 + NHB2].unsqueeze(2).broadcast_to([128, NHB2, B])
                        for ccc in range(C):
                            xr, xi = X_re4[:, :, :, ccc], X_im4[:, :, :, ccc]
                            v.tensor_add(t_r3, Sr3, xr)
                            gp.tensor_add(t_i3, Si3, xi)
                            sc.activation(xr, Sr3, AF.Copy)
                            sc.activation(xi, Si3, AF.Copy)
                            v.tensor_mul(w13, t_r3, dreb)
                            v.tensor_mul(w23, t_i3, dimb)
                            v.tensor_sub(Sr3, w13, w23)
                            v.tensor_mul(w13, t_i3, dreb)
                            v.tensor_mul(w23, t_r3, dimb)
                            v.tensor_add(Si3, w13, w23)

                    # E table for this half
                    gp_ps = hsc.enter_context(tc.tile_pool(name=f"gps{l}{half}", bufs=2, space="PSUM"))
                    cv_ps = hsc.enter_context(tc.tile_pool(name=f"cvps{l}{half}", bufs=2, space="PSUM"))
                    tpe = hsc.enter_context(tc.tile_pool(name=f"tabE{l}{half}", bufs=1))
                    E_re = tpe.tile([128, NHB2 * T], F32, tag="E_re", name="E_re")
                    E_im = tpe.tile([128, NHB2 * T], F32, tag="E_im", name="E_im")
                    E16re = tpe.tile([128, NHB2 * T], BF16, tag="E16re", name="E16re")
                    E16im = tpe.tile([128, NHB2 * T], BF16, tag="E16im", name="E16im")
                    with tc.tile_pool(name=f"dblE{l}{half}", bufs=1) as dwk:
                        _build_table(v, E_re, E_im,
                                     esr[:, hb0 : hb0 + NHB2], esi[:, hb0 : hb0 + NHB2],
                                     wre[:, hb0 : hb0 + NHB2], wim[:, hb0 : hb0 + NHB2],
                                     dwk, NHB2)
                    v.tensor_copy(E16re[:], E_re[:])
                    v.tensor_copy(E16im[:], E_im[:])

                    # per-h conv
                    yg = [None, None]
                    for hh in range(HS // 2):
                        h = 2 * hb0 + hh
                        hp, hb = h & 1, h >> 1
                        base = 64 * hp
                        kb = hb - hb0
                        er = E16re[base : base + 64, kb * T : (kb + 1) * T]
                        ei = E16im[base : base + 64, kb * T : (kb + 1) * T]
                        rr = R16re[base : base + 64, kb * T : (kb + 1) * T]
                        ri = R16im[base : base + 64, kb * T : (kb + 1) * T]
                        psG = gp_ps.tile([128, T], F32, tag="psG", name="psG")
                        te.matmul(psG[:], rr, er, start=True, stop=False)
                        te.matmul(psG[:], ri, ei, start=False, stop=True)
                        GTt = gt_pool.tile([128, T], F32, tag="GTt", name="GTt")
                        GT = gt_pool.tile([128, T], BF16, tag="GT", name="GT")
                        v.tensor_mul(GTt[:], psG[:], trimask[:])
                        v.scalar_tensor_tensor(GT[:], ident[:], drep[:, h : h + 1], GTt[:],
                                               op0=OP.mult, op1=OP.add)
                        if hh % HG == 0:
                            yg = [ya_pool.tile([128, HG * T], BF16, tag=f"yg{qq}", name=f"yg{qq}")
                                  for qq in (0, 1)]
                        for qq in (0, 1):
                            ps = cv_ps.tile([128, T], F32, tag="ps", name="ps")
                            lu = u_slice(h, qq)
                            te.matmul(ps[:], lu, GT[:], start=True, stop=False)
                            lr = X_re[base : base + 64,
                                      kb * B * C + qq * 128 : kb * B * C + qq * 128 + 128]
                            li = X_im[base : base + 64,
                                      kb * B * C + qq * 128 : kb * B * C + qq * 128 + 128]
                            te.matmul(ps[:], lr, er, start=False, stop=False)
                            te.matmul(ps[:], li, ei, start=False, stop=True)
                            sc.activation(yg[qq][:, (hh % HG) * T : (hh % HG + 1) * T], ps[:],
                                          AF.Gelu_apprx_tanh)
                        if hh % HG == HG - 1:
                            hg0 = h - HG + 1
                            for qq in (0, 1):
                                ygv = yg[qq][:].rearrange("bc (hh2 j) -> bc hh2 j", j=T)
                                for dd in range(4):
                                    d = qq * 4 + dd
                                    dst = a2a_y_in[l][d, hg0 : hg0 + HG, :, :].rearrange(
                                        "hh2 b2 (c j) -> (b2 c) hh2 j", j=T)
                                    nc.sync.dma_start(dst, ygv[32 * dd : 32 * dd + 32, :, :])

            # ======== AllToAll y ========
            gp.collective_compute(
                "AllToAll", OP.bypass, replica_groups=RG,
                ins=[a2a_y_in[l][:].opt()], outs=[a2a_y_out[l][:].opt()])

            # ======== GLU PHASE (B-shard) ========
            with contextlib.ExitStack() as gl:
                gpool = gl.enter_context(tc.tile_pool(name=f"glu{l}", bufs=1))
                wtiles = [gpool.tile([128, 2 * H], BF16, tag=f"wt{k}", name=f"wt{k}") for k in range(4)]
                ytiles = [gpool.tile([128, B2 * L], BF16, tag=f"yk{k}", name=f"yk{k}") for k in range(4)]
                for kt in range(4):
                    nc.sync.dma_start(wtiles[kt][:], par_in[("wt", l)][128 * kt : 128 * (kt + 1), :])
                    src = a2a_y_out[l][:].rearrange("s h b2 ll -> (s h) (b2 ll)")
                    nc.sync.dma_start(ytiles[kt][:], src[128 * kt : 128 * (kt + 1), :])
                if debug and l == 0:
                    for s in range(CORES):
                        gb = gpool.tile([64, B2 * L], BF16, tag="dbgy", name="dbgy")
                        nc.sync.dma_start(gb[:], a2a_y_out[l][s].rearrange("h b2 ll -> h (b2 ll)"))
                        nc.sync.dma_start(dbg["yact0"][s].rearrange("h b2 ll -> h (b2 ll)"), gb[:])
                zps = gl.enter_context(tc.tile_pool(name=f"zps{l}", bufs=2, space="PSUM"))
                if l == 0:
                    zwp = gl.enter_context(tc.tile_pool(name=f"zw{l}", bufs=3))
                    ubp = gl.enter_context(tc.tile_pool(name=f"ub{l}", bufs=3))
                    tps = gl.enter_context(tc.tile_pool(name=f"tps{l}", bufs=2, space="PSUM"))
                    trp = gl.enter_context(tc.tile_pool(name=f"trp{l}", bufs=3))
                    u0bf = u0b_in[:].rearrange("ch b2 ll -> ch (b2 ll)")
                    for kt in range(4):
                        for ch in range(8):
                            sl = slice(ch * 512, (ch + 1) * 512)
                            psZ = zps.tile([128, 512], F32, tag="psZ", name="psZ")
                            for k2 in range(4):
                                te.matmul(psZ[:],
                                          wtiles[k2][:, kt * 128 : (kt + 1) * 128],
                                          ytiles[k2][:, sl],
                                          start=(k2 == 0), stop=(k2 == 3))
                            z1c = zwp.tile([128, 512], F32, tag="z1c", name="z1c")
                            v.tensor_scalar(z1c[:], psZ[:], brep0[:, kt : kt + 1], None, op0=OP.add)
                            psZ2 = zps.tile([128, 512], F32, tag="psZ2", name="psZ2")
                            for k2 in range(4):
                                te.matmul(psZ2[:],
                                          wtiles[k2][:, (kt + 4) * 128 : (kt + 5) * 128],
                                          ytiles[k2][:, sl],
                                          start=(k2 == 0), stop=(k2 == 3))
                            sgc = zwp.tile([128, 512], F32, tag="sgc", name="sgc")
                            sc.activation(sgc[:], psZ2[:], AF.Sigmoid, bias=brep0[:, kt + 4 : kt + 5])
                            ub = ubp.tile([128, 512], F32, tag="ub", name="ub")
                            nc.sync.dma_start(ub[:], u0bf[128 * kt : 128 * (kt + 1), sl])
                            v.tensor_mul(z1c[:], z1c[:], sgc[:])
                            v.tensor_add(z1c[:], z1c[:], ub[:])
                            # transpose the 4 l-tiles of this chunk and send
                            b2c = ch // 4
                            for c4 in range(4):
                                ccc = (ch % 4) * 4 + c4
                                psT = tps.tile([128, 128], F32, tag="psT2", name="psT2")
                                te.transpose(psT[:], z1c[:, c4 * 128 : (c4 + 1) * 128], ident[:])
                                trsb = trp.tile([128, 128], BF16, tag="trsb", name="trsb")
                                sc.activation(trsb[:], psT[:], AF.Copy)
                                dst = a2a_u_in[:, :, b2c, ccc, :][2 * kt : 2 * kt + 2].rearrange(
                                    "e j hh -> j e hh")
                                nc.sync.dma_start(dst, trsb[:].rearrange("j (e hh) -> j e hh", hh=64))
                    gp.collective_compute(
                        "AllToAll", OP.bypass, replica_groups=RG,
                        ins=[a2a_u_in[:].opt()], outs=[a2a_u_out[:].opt()])
                    for s in range(CORES):
                        src = a2a_u_out[s].rearrange("j b2 c h -> j (b2 c h)")
                        dstv = u_sb[:].rearrange("j (b c h) -> j b c h", b=B, c=C)[
                            :, 2 * s : 2 * s + 2, :, :].rearrange("j b c h -> j (b c h)")
                        nc.sync.dma_start(dstv, src)
                    if debug:
                        for jj in range(2):
                            nc.sync.dma_start(
                                dbg["u1"][64 * jj : 64 * jj + 64].rearrange("j b c h -> j (b c h)"),
                                u_sb[64 * jj : 64 * jj + 64, :])
                else:
                    zw1 = gl.enter_context(tc.tile_pool(name=f"zw1{l}", bufs=3))
                    b1b = gpool.tile([128, 2 * H], F32, tag="b1b", name="b1b")
                    nc.sync.dma_start(b1b[:], b1row_in[:].broadcast_to([128, 2 * H]))
                    for b2 in range(B2):
                        for lt in range(C):
                            zz = []
                            for oh in (0, 1):
                                psW = zps.tile([128, 512], F32, tag="psW", name="psW")
                                for kt in range(4):
                                    te.matmul(psW[:],
                                              ytiles[kt][:, b2 * L + lt * T : b2 * L + (lt + 1) * T],
                                              wtiles[kt][:, oh * 512 : (oh + 1) * 512],
                                              start=(kt == 0), stop=(kt == 3))
                                zt = zw1.tile([128, 512], F32, tag=f"zt{oh}", name=f"zt{oh}")
                                v.tensor_add(zt[:], psW[:], b1b[:, oh * 512 : (oh + 1) * 512])
                                zz.append(zt)
                            sg = zw1.tile([128, 512], F32, tag="sg1", name="sg1")
                            sc.activation(sg[:], zz[1][:], AF.Sigmoid)
                            osb = zw1.tile([128, 512], F32, tag="osb", name="osb")
                            v.tensor_mul(osb[:], zz[0][:], sg[:])
                            nc.sync.dma_start(out_z[b2, lt * T : (lt + 1) * T, :], osb[:])
    nc.finalize()
    _NC_CACHE[key] = nc
    return nc


# ====================== host side ======================

def _prep_core_inputs(core, x, pars):
    hs = slice(HS * core, HS * (core + 1))
    ins = {}
    xs = x[:, :, hs]                                    # (B, L, 64)
    u0 = xs.reshape(B, C, T, HS).transpose(2, 0, 1, 3)  # (j, b, c, h)
    ins["u0"] = np.ascontiguousarray(u0)
    xb = x[B2 * core : B2 * (core + 1)]                 # (2, L, H)
    ins["u0b"] = np.ascontiguousarray(xb.transpose(2, 0, 1))
    ins["trimask"] = np.triu(np.ones((T, T), np.float32))
    ins["ident"] = np.eye(T, dtype=np.float32)

    def scan_layout(a):
        if a.ndim == 1:
            a = np.broadcast_to(a[:, None], (HS, N))
        return np.ascontiguousarray(
            a.reshape(NHB, 2, N).transpose(1, 2, 0).reshape(128, NHB))

    for l in (0, 1):
        ins[f"ldt{l}"] = scan_layout(pars[f"ldt{l}"][hs])
        ins[f"lare{l}"] = scan_layout(pars[f"lAre{l}"][hs])
        ins[f"aim{l}"] = scan_layout(pars[f"Aim{l}"][hs])
        ins[f"cre{l}"] = scan_layout(pars[f"Cre{l}"][hs])
        ins[f"cim{l}"] = scan_layout(pars[f"Cim{l}"][hs])
        ins[f"drep{l}"] = np.ascontiguousarray(
            np.broadcast_to(pars[f"D{l}"][hs][None, :], (128, HS)))
        ins[f"wt{l}"] = np.ascontiguousarray(pars[f"W{l}"].T)
    ins["brep0"] = np.ascontiguousarray(pars["b0"].reshape(8, 128).T)
    ins["b1row"] = np.ascontiguousarray(pars["b1"][None, :])
    out = {k: vv.astype(np.float32) for k, vv in ins.items()}
    out["u0"] = ins["u0"].astype(ml_dtypes.bfloat16)
    for l in (0, 1):
        out[f"wt{l}"] = ins[f"wt{l}"].astype(ml_dtypes.bfloat16)
    return out


def run(x, pars, debug=False, trace=False):
    nc = build_kernel(debug=debug)
    in_maps = [_prep_core_inputs(c, x, pars) for c in range(CORES)]
    r = run_bass_kernel_spmd(nc, in_maps, core_ids=list(range(CORES)), trace=trace)
    outs = np.stack([r.results[c]["out"] for c in range(CORES)])  # (8, 2, L, H)
    full = outs.reshape(B, L, H)
    return full, r


def kernel(**inputs):
    x = np.asarray(inputs["x"], dtype=np.float32)
    pars = {k: np.asarray(vv, dtype=np.float32) for k, vv in inputs.items() if k != "x"}
    full, _ = run(x, pars)
    return full



# revision 27
# speedup vs baseline: 1.8456x; 1.1469x over previous
"""Trainium2 Bass kernel for the 2-layer S4D block (nn_MetaS4History).

Strategy (8 cores, single launch):
  - Conv phases H-sharded (64 channels/core, full batch): chunked-SSD convolution
    with T=128 chunks: per-h matmuls (G-build, intra, injection) + a 16-step
    DVE scan for inter-chunk states. Matmuls in bf16 (PSUM fp32 accumulate).
  - GLU projections B-sharded (2 batch elems/core, full H): y-stationary bf16
    matmuls producing [l-pos, 2H] tiles (no output transposes needed).
  - Phase boundaries resharded with AllToAll collectives (3 total, bf16).
Layouts: u_sb [j,(h,b,c)] (contiguous matmul operands); X states
[128,(c,hb,b)] (contiguous scan); zstage [j,(h,c)] (contiguous a2a DMAs).
"""
import contextlib
import ml_dtypes
import numpy as np
import concourse.bacc as bacc
import concourse.mybir as mybir
from concourse.tile import TileContext
from concourse.bass_utils import run_bass_kernel_spmd

F32 = mybir.dt.float32
BF16 = mybir.dt.bfloat16
AF = mybir.ActivationFunctionType
OP = mybir.AluOpType

CORES = 8
B, L, H, N = 16, 2048, 512, 64
T, C = 128, 16          # chunk len, chunk count
HS = H // CORES         # 64 channels per core
B2 = B // CORES         # 2 batch per core (GLU phase)
NHB = HS // 2           # 32 h-blocks (h = 2*hblk + hpar)
RG = [list(range(CORES))]

_NC_CACHE = {}


def _build_table(eng, tre, tim, seed_re, seed_im, mul_re, mul_im, wk, nhb):
    """Power table via doubling: tab[:, hb, j] = seed * mul^j, j in 0..T-1.
    tre/tim: [128, nhb*T] tiles; seed/mul: [128, nhb] APs (sliced); wk: pool."""
    t3re = tre[:].rearrange("p (h j) -> p h j", j=T)
    t3im = tim[:].rearrange("p (h j) -> p h j", j=T)
    eng.tensor_copy(t3re[:, :, 0:1], seed_re.unsqueeze(2))
    eng.tensor_copy(t3im[:, :, 0:1], seed_im.unsqueeze(2))
    mre = wk.tile([128, nhb], F32, tag="dbl_mre", name="dbl_mre")
    mim = wk.tile([128, nhb], F32, tag="dbl_mim", name="dbl_mim")
    q1 = wk.tile([128, nhb], F32, tag="dbl_q1", name="dbl_q1")
    q2 = wk.tile([128, nhb], F32, tag="dbl_q2", name="dbl_q2")
    sc1 = wk.tile([128, nhb * T // 2], F32, tag="dbl_s1", name="dbl_s1")
    eng.tensor_copy(mre[:], mul_re)
    eng.tensor_copy(mim[:], mul_im)
    m = 1
    while m < T:
        mbre = mre[:].unsqueeze(2).broadcast_to([128, nhb, m])
        mbim = mim[:].unsqueeze(2).broadcast_to([128, nhb, m])
        s1 = sc1[:].rearrange("p (h j) -> p h j", j=T // 2)[:, :, 0:m]
        src_re, src_im = t3re[:, :, 0:m], t3im[:, :, 0:m]
        dst_re, dst_im = t3re[:, :, m : 2 * m], t3im[:, :, m : 2 * m]
        eng.tensor_mul(s1, src_im, mbim)
        eng.tensor_mul(dst_re, src_re, mbre)
        eng.tensor_sub(dst_re, dst_re, s1)
        eng.tensor_mul(s1, src_im, mbre)
        eng.tensor_mul(dst_im, src_re, mbim)
        eng.tensor_add(dst_im, dst_im, s1)
        m *= 2
        if m < T:
            eng.tensor_mul(q1[:], mre[:], mre[:])
            eng.tensor_mul(q2[:], mim[:], mim[:])
            eng.tensor_mul(mim[:], mre[:], mim[:])
            eng.tensor_add(mim[:], mim[:], mim[:])
            eng.tensor_sub(mre[:], q1[:], q2[:])


def build_kernel(debug=False):
    key = debug
    if key in _NC_CACHE:
        return _NC_CACHE[key]
    nc = bacc.Bacc(num_devices=CORES)
    v = nc.vector
    gp = nc.gpsimd
    sc = nc.scalar
    te = nc.tensor

    # ---------------- DRAM I/O ----------------
    u0_in = nc.dram_tensor("u0", [T, HS, B, C], BF16, kind="ExternalInput")
    u0b_in = nc.dram_tensor("u0b", [B2, C, T, H], F32, kind="ExternalInput")
    trimask_in = nc.dram_tensor("trimask", [T, T], F32, kind="ExternalInput")
    ident_in = nc.dram_tensor("ident", [T, T], F32, kind="ExternalInput")
    par_in = {}
    for l in (0, 1):
        for nm in ("ldt", "lare", "aim", "cre", "cim"):
            par_in[(nm, l)] = nc.dram_tensor(f"{nm}{l}", [128, NHB], F32, kind="ExternalInput")
        par_in[("drep", l)] = nc.dram_tensor(f"drep{l}", [128, HS], F32, kind="ExternalInput")
        par_in[("wt", l)] = nc.dram_tensor(f"wt{l}", [H, 2 * H], BF16, kind="ExternalInput")
        par_in[("brow", l)] = nc.dram_tensor(f"brow{l}", [1, 2 * H], F32, kind="ExternalInput")

    a2a_y_in = [nc.dram_tensor(f"a2aY{l}_in", [CORES, HS, B2, L], BF16) for l in (0, 1)]
    a2a_y_out = [nc.dram_tensor(f"a2aY{l}_out", [CORES, HS, B2, L], BF16) for l in (0, 1)]
    a2a_u_in = nc.dram_tensor("a2aU_in", [CORES, T, B2, HS, C], BF16)
    a2a_u_out = nc.dram_tensor("a2aU_out", [CORES, T, B2, HS, C], BF16)
    out_z = nc.dram_tensor("out", [B2, L, H], F32, kind="ExternalOutput")

    with TileContext(nc) as tc, contextlib.ExitStack() as top:
        cpool = top.enter_context(tc.tile_pool(name="consts", bufs=1))
        trimask = cpool.tile([T, T], F32, tag="trimask", name="trimask")
        ident = cpool.tile([T, T], F32, tag="ident", name="ident")
        nc.sync.dma_start(trimask[:], trimask_in[:])
        nc.sync.dma_start(ident[:], ident_in[:])
        csts = cpool.tile([128, 32], F32, tag="csts", name="csts")
        SINC = [1.0, -1.0 / 6, 1.0 / 120, -1.0 / 5040, 1.0 / 362880, -1.0 / 39916800]
        COSC = [1.0, -1.0 / 2, 1.0 / 24, -1.0 / 720, 1.0 / 40320, -1.0 / 3628800]
        for k in range(6):
            nc.any.memset(csts[:, k : k + 1], SINC[k])
            nc.any.memset(csts[:, 6 + k : 7 + k], COSC[k])
        nc.any.memset(csts[:, 12:13], -1.0)
        nc.any.memset(csts[:, 13:14], 2.0)
        nc.any.memset(csts[:, 14:15], 1.0 / 16)
        import math
        for k in range(11):
            nc.any.memset(csts[:, 16 + k : 17 + k], 1.0 / math.factorial(k))
        nc.any.memset(csts[:, 27:28], 1.0 / 8)

        upool = top.enter_context(tc.tile_pool(name="u", bufs=1))
        u_sb = upool.tile([T, HS * B * C], BF16, tag="u_sb", name="u_sb")  # [j,(h,b,c)]
        nc.sync.dma_start(u_sb[:], u0_in[:].rearrange("j h b c -> j (h b c)"))

        def u_slice(h, bq=None):
            b0, nb = (0, B) if bq is None else (bq * 8, 8)
            ap = u_sb[:].rearrange("j (h b c) -> j h b c", b=B, c=C)
            return ap[:, h, b0 : b0 + nb, :]

        for l in (0, 1):
            # ======== CONV PHASE (H-shard) ========
            with contextlib.ExitStack() as cv:
                pp = cv.enter_context(tc.tile_pool(name=f"par{l}", bufs=1))
                P = {}
                for nm in ("ldt", "lare", "aim", "cre", "cim"):
                    P[nm] = pp.tile([128, NHB], F32, tag=f"p_{nm}", name=f"p_{nm}")
                    nc.sync.dma_start(P[nm][:], par_in[(nm, l)][:])
                drep = pp.tile([128, HS], F32, tag="p_drep", name="p_drep")
                nc.sync.dma_start(drep[:], par_in[("drep", l)][:])

                def wk(tag):
                    return pp.tile([128, NHB], F32, tag=tag, name=tag)[:]

                neg1 = csts[:, 12:13]
                two = csts[:, 13:14]
                s16 = csts[:, 14:15]

                def exp_poly(out, x):
                    """out = e^x via (T10(x/8))^8; |x| <= 8. Accurate to ~1e-7."""
                    ea = wk("exp_a")
                    et = wk("exp_t")
                    v.tensor_scalar(ea, x, csts[:, 27:28], None, op0=OP.mult)  # y = x/8
                    v.tensor_scalar(et, ea, csts[:, 26:27], csts[:, 25:26], op0=OP.mult, op1=OP.add)
                    for k in range(8, -1, -1):
                        v.tensor_mul(et, et, ea)
                        v.tensor_scalar(et, et, csts[:, 16 + k : 17 + k], None, op0=OP.add)
                    for _ in range(3):
                        v.tensor_mul(et, et, et)
                    v.tensor_copy(out, et)

                dt, eA = wk("dt"), wk("eA")
                exp_poly(dt, P["ldt"][:])
                exp_poly(eA, P["lare"][:])
                dtAre, dtAim = wk("dtAre"), wk("dtAim")
                v.scalar_tensor_tensor(dtAre, dt, -1.0, eA, op0=OP.mult, op1=OP.mult)
                v.tensor_mul(dtAim, dt, P["aim"][:])
                mag = wk("mag")
                exp_poly(mag, dtAre)
                q, x2 = wk("q"), wk("x2")
                v.tensor_scalar(q, dtAim, s16, None, op0=OP.mult)
                v.tensor_mul(x2, q, q)
                acc, t1, t2 = wk("acc"), wk("t1"), wk("t2")
                cr, ci = wk("cr"), wk("ci")
                v.tensor_scalar(acc, x2, csts[:, 5:6], csts[:, 4:5], op0=OP.mult, op1=OP.add)
                for k in (3, 2, 1, 0):
                    v.tensor_mul(t1, acc, x2)
                    v.tensor_scalar(acc, t1, csts[:, k : k + 1], None, op0=OP.add)
                v.tensor_mul(ci, acc, q)
                v.tensor_scalar(acc, x2, csts[:, 11:12], csts[:, 10:11], op0=OP.mult, op1=OP.add)
                for k in (9, 8, 7, 6):
                    v.tensor_mul(t1, acc, x2)
                    v.tensor_scalar(acc, t1, csts[:, k : k + 1], None, op0=OP.add)
                v.tensor_copy(cr, acc)
                for _ in range(4):
                    v.tensor_mul(t1, cr, cr)
                    v.tensor_mul(t2, ci, ci)
                    v.scalar_tensor_tensor(acc, cr, 2.0, ci, op0=OP.mult, op1=OP.mult)
                    v.tensor_sub(cr, t1, t2)
                    v.tensor_copy(ci, acc)
                wre, wim = wk("wre"), wk("wim")
                v.tensor_mul(wre, mag, cr)
                v.tensor_mul(wim, mag, ci)
                m2, im2 = wk("m2"), wk("im2")
                v.tensor_mul(m2, mag, mag)
                v.reciprocal(im2, m2)
                rpre, rpim = wk("rpre"), wk("rpim")
                v.tensor_mul(rpre, wre, im2)
                v.tensor_mul(rpim, wim, im2)
                wm1re = wk("wm1re")
                v.tensor_scalar(wm1re, wre, neg1, None, op0=OP.add)
                tre, tim = wk("tre"), wk("tim")
                v.tensor_mul(t1, P["cre"][:], wm1re)
                v.tensor_mul(t2, P["cim"][:], wim)
                v.tensor_sub(tre, t1, t2)
                v.tensor_mul(t1, P["cre"][:], wim)
                v.tensor_mul(t2, P["cim"][:], wm1re)
                v.tensor_add(tim, t1, t2)
                den, invd = wk("den"), wk("invd")
                v.tensor_mul(t1, eA, eA)
                v.tensor_mul(t2, P["aim"][:], P["aim"][:])
                v.tensor_add(den, t1, t2)
                v.reciprocal(invd, den)
                ccr, cci = wk("ccr"), wk("cci")
                v.tensor_mul(t1, tre, eA)
                v.tensor_mul(t2, tim, P["aim"][:])
                v.tensor_sub(acc, t2, t1)
                v.tensor_mul(ccr, acc, invd)
                v.tensor_mul(t1, tre, P["aim"][:])
                v.tensor_mul(t2, tim, eA)
                v.tensor_add(acc, t1, t2)
                v.tensor_mul(t1, acc, invd)
                v.tensor_scalar(cci, t1, neg1, None, op0=OP.mult)
                esr, esi = wk("esr"), wk("esi")
                v.tensor_mul(t1, ccr, wre)
                v.tensor_mul(t2, cci, wim)
                v.tensor_sub(acc, t1, t2)
                v.tensor_scalar(esr, acc, two, None, op0=OP.mult)
                v.tensor_mul(t1, ccr, wim)
                v.tensor_mul(t2, cci, wre)
                v.tensor_add(acc, t1, t2)
                v.tensor_scalar(esi, acc, two, None, op0=OP.mult)
                wtr, wti = wk("wtr"), wk("wti")
                v.tensor_copy(wtr, wre)
                v.tensor_copy(wti, wim)
                for _ in range(7):
                    v.tensor_mul(t1, wtr, wtr)
                    v.tensor_mul(t2, wti, wti)
                    v.scalar_tensor_tensor(acc, wtr, 2.0, wti, op0=OP.mult, op1=OP.mult)
                    v.tensor_sub(wtr, t1, t2)
                    v.tensor_copy(wti, acc)
                dre, dim_ = wk("dre"), wk("dim")
                v.tensor_copy(dre, wtr)
                v.tensor_scalar(dim_, wti, neg1, None, op0=OP.mult)

                # ---------- full-width tables ----------
                tpr = cv.enter_context(tc.tile_pool(name=f"tabR{l}", bufs=1))
                Rp_re = tpr.tile([128, NHB * T], F32, tag="Rp_re", name="Rp_re")
                Rp_im = tpr.tile([128, NHB * T], F32, tag="Rp_im", name="Rp_im")
                R16re = tpr.tile([128, NHB * T], BF16, tag="R16re", name="R16re")
                R16im = tpr.tile([128, NHB * T], BF16, tag="R16im", name="R16im")
                with tc.tile_pool(name=f"dblR{l}", bufs=1) as dwk:
                    _build_table(gp, Rp_re, Rp_im, rpre[:], rpim[:],
                                 rpre[:], rpim[:], dwk, NHB)
                sc.activation(R16re[:], Rp_re[:], AF.Copy)
                v.tensor_copy(R16im[:], Rp_im[:])

                tpe = cv.enter_context(tc.tile_pool(name=f"tabE{l}", bufs=1))
                E_re = tpe.tile([128, NHB * T], F32, tag="E_re", name="E_re")
                E_im = tpe.tile([128, NHB * T], F32, tag="E_im", name="E_im")
                E16re = tpe.tile([128, NHB * T], BF16, tag="E16re", name="E16re")
                E16im = tpe.tile([128, NHB * T], BF16, tag="E16im", name="E16im")
                with tc.tile_pool(name=f"dblE{l}", bufs=1) as dwk:
                    _build_table(v, E_re, E_im, esr[:], esi[:],
                                 wre[:], wim[:], dwk, NHB)
                sc.activation(E16re[:], E_re[:], AF.Copy)
                v.tensor_copy(E16im[:], E_im[:])

                # ---------- collection: X[p, (hb, b, c)] chunk states ----------
                stp = cv.enter_context(tc.tile_pool(name=f"st{l}", bufs=1))
                X_re = stp.tile([128, NHB * B * C], BF16, tag="X_re", name="X_re")
                X_im = stp.tile([128, NHB * B * C], BF16, tag="X_im", name="X_im")
                X_re4 = X_re[:].rearrange("p (h b c) -> p h b c", b=B, c=C)
                X_im4 = X_im[:].rearrange("p (h b c) -> p h b c", b=B, c=C)

                with tc.tile_pool(name=f"wsl{l}", bufs=3) as wslp, \
                     tc.tile_pool(name=f"pst{l}", bufs=2, space="PSUM") as pstp, \
                     tc.tile_pool(name=f"psc{l}", bufs=2, space="PSUM") as pscp:
                    for k in range(NHB):
                        wsl = [wslp.tile([128, T], BF16, tag=f"wsl{comp}", name=f"wsl{comp}")
                               for comp in (0, 1)]
                        for comp, Rt in enumerate((Rp_re, Rp_im)):
                            psT = pstp.tile([128, T], F32, tag="psT", name="psT")
                            te.transpose(psT[:], Rt[:, k * T : (k + 1) * T], ident[:])
                            sc.activation(wsl[comp][:], psT[:], AF.Copy)
                        psr = pscp.tile([128, B * C], F32, tag="psr", name="psr")
                        psi = pscp.tile([128, B * C], F32, tag="psi", name="psi")
                        for hp in (0, 1):
                            h = 2 * k + hp
                            us = u_slice(h)
                            te.matmul(psr[64 * hp : 64 * hp + 64, :],
                                      wsl[0][:, 64 * hp : 64 * hp + 64], us, start=True, stop=True)
                            te.matmul(psi[64 * hp : 64 * hp + 64, :],
                                      wsl[1][:, 64 * hp : 64 * hp + 64], us, start=True, stop=True)
                        sc.activation(X_re4[:, k, :, :], psr[:], AF.Copy)
                        sc.activation(X_im4[:, k, :, :], psi[:], AF.Copy)

                # ---------- scan (in place: X becomes S_before) ----------
                with tc.tile_pool(name=f"scan{l}", bufs=1) as sp:
                    def stile(nm):
                        return sp.tile([128, NHB * B], F32, tag=nm, name=nm)[:].rearrange(
                            "p (h b) -> p h b", b=B)
                    Sr3, Si3 = stile("Sr"), stile("Si")
                    t_r3, t_i3 = stile("tm_r"), stile("tm_i")
                    w1r, w2r = stile("w1r"), stile("w2r")
                    w1i, w2i = stile("w1i"), stile("w2i")
                    nc.any.memset(Sr3, 0.0)
                    nc.any.memset(Si3, 0.0)
                    dreb = dre[:].unsqueeze(2).broadcast_to([128, NHB, B])
                    dimb = dim_[:].unsqueeze(2).broadcast_to([128, NHB, B])
                    for ccc in range(C):
                        xr, xi = X_re4[:, :, :, ccc], X_im4[:, :, :, ccc]
                        v.tensor_add(t_r3, Sr3, xr)
                        gp.tensor_add(t_i3, Si3, xi)
                        sc.activation(xr, Sr3, AF.Copy)
                        sc.activation(xi, Si3, AF.Copy)
                        v.tensor_mul(w1r, t_r3, dreb)
                        v.tensor_mul(w2r, t_i3, dimb)
                        v.tensor_sub(Sr3, w1r, w2r)
                        gp.tensor_mul(w1i, t_i3, dreb)
                        gp.tensor_mul(w2i, t_r3, dimb)
                        gp.tensor_add(Si3, w1i, w2i)

                # ---------- per-h conv ----------
                gt_pool = cv.enter_context(tc.tile_pool(name=f"gt{l}", bufs=3))
                ya_pool = cv.enter_context(tc.tile_pool(name=f"ya{l}", bufs=2))
                gp_ps = cv.enter_context(tc.tile_pool(name=f"gps{l}", bufs=2, space="PSUM"))
                cv_ps = cv.enter_context(tc.tile_pool(name=f"cvps{l}", bufs=2, space="PSUM"))
                HG = 8
                yg = [None, None]
                for h in range(HS):
                    hp, hb = h & 1, h >> 1
                    base = 64 * hp
                    er = E16re[base : base + 64, hb * T : (hb + 1) * T]
                    ei = E16im[base : base + 64, hb * T : (hb + 1) * T]
                    rr = R16re[base : base + 64, hb * T : (hb + 1) * T]
                    ri = R16im[base : base + 64, hb * T : (hb + 1) * T]
                    psG = gp_ps.tile([128, T], F32, tag="psG", name="psG")
                    te.matmul(psG[:], rr, er, start=True, stop=False)
                    te.matmul(psG[:], ri, ei, start=False, stop=True)
                    GTt = gt_pool.tile([128, T], F32, tag="GTt", name="GTt")
                    GT = gt_pool.tile([128, T], BF16, tag="GT", name="GT")
                    v.tensor_mul(GTt[:], psG[:], trimask[:])
                    v.scalar_tensor_tensor(GT[:], ident[:], drep[:, h : h + 1], GTt[:],
                                           op0=OP.mult, op1=OP.add)
                    if h % HG == 0:
                        yg = [ya_pool.tile([128, HG * T], BF16, tag=f"yg{qq}", name=f"yg{qq}")
                              for qq in (0, 1)]
                    for qq in (0, 1):
                        ps = cv_ps.tile([128, T], F32, tag="ps", name="ps")
                        lu = u_slice(h, qq)
                        te.matmul(ps[:], lu, GT[:], start=True, stop=False)
                        lr = X_re[base : base + 64,
                                  hb * B * C + qq * 128 : hb * B * C + qq * 128 + 128]
                        li = X_im[base : base + 64,
                                  hb * B * C + qq * 128 : hb * B * C + qq * 128 + 128]
                        te.matmul(ps[:], lr, er, start=False, stop=False)
                        te.matmul(ps[:], li, ei, start=False, stop=True)
                        sc.activation(yg[qq][:, (h % HG) * T : (h % HG + 1) * T], ps[:],
                                      AF.Gelu_apprx_tanh)
                    if h % HG == HG - 1:
                        hg0 = h - HG + 1
                        for qq in (0, 1):
                            ygv = yg[qq][:].rearrange("bc (hh2 j) -> bc hh2 j", j=T)
                            for dd in range(4):
                                d = qq * 4 + dd
                                dst = a2a_y_in[l][d, hg0 : hg0 + HG, :, :].rearrange(
                                    "hh2 b2 (c j) -> (b2 c) hh2 j", j=T)
                                nc.sync.dma_start(dst, ygv[32 * dd : 32 * dd + 32, :, :])

            # ======== AllToAll y ========
            gp.collective_compute(
                "AllToAll", OP.bypass, replica_groups=RG,
                ins=[a2a_y_in[l][:].opt()], outs=[a2a_y_out[l][:].opt()])

            # ======== GLU PHASE (B-shard) ========
            with contextlib.ExitStack() as gl:
                gpool = gl.enter_context(tc.tile_pool(name=f"glu{l}", bufs=1))
                wtiles = [gpool.tile([128, 2 * H], BF16, tag=f"wt{k}", name=f"wt{k}") for k in range(4)]
                ytiles = [gpool.tile([128, B2 * L], BF16, tag=f"yk{k}", name=f"yk{k}") for k in range(4)]
                for kt in range(4):
                    nc.sync.dma_start(wtiles[kt][:], par_in[("wt", l)][128 * kt : 128 * (kt + 1), :])
                    src = a2a_y_out[l][:].rearrange("s h b2 ll -> (s h) (b2 ll)")
                    nc.sync.dma_start(ytiles[kt][:], src[128 * kt : 128 * (kt + 1), :])
                bb = gpool.tile([128, 2 * H], F32, tag="bb", name="bb")
                nc.sync.dma_start(bb[:], par_in[("brow", l)][:].broadcast_to([128, 2 * H]))
                if l == 0:
                    zstage = [gpool.tile([128, H * C], BF16, tag=f"zst{b2}", name=f"zst{b2}")
                              for b2 in range(B2)]
                    zs3 = [z[:].rearrange("p (h c) -> p h c", c=C) for z in zstage]
                zps = gl.enter_context(tc.tile_pool(name=f"zps{l}", bufs=2, space="PSUM"))
                zwp = gl.enter_context(tc.tile_pool(name=f"zw{l}", bufs=3))
                ubp = gl.enter_context(tc.tile_pool(name=f"ub{l}", bufs=3))
                for b2 in range(B2):
                    for ct in range(C):
                        psZ1 = zps.tile([128, H], F32, tag="psZ1", name="psZ1")
                        psZ2 = zps.tile([128, H], F32, tag="psZ2", name="psZ2")
                        for kt in range(4):
                            yst = ytiles[kt][:, b2 * L + ct * T : b2 * L + (ct + 1) * T]
                            te.matmul(psZ1[:], yst, wtiles[kt][:, :H],
                                      start=(kt == 0), stop=(kt == 3))
                            te.matmul(psZ2[:], yst, wtiles[kt][:, H:],
                                      start=(kt == 0), stop=(kt == 3))
                        zb = zwp.tile([128, 2 * H], F32, tag="zb", name="zb")
                        v.tensor_add(zb[:, :H], psZ1[:], bb[:, :H])
                        v.tensor_add(zb[:, H:], psZ2[:], bb[:, H:])
                        sg = zwp.tile([128, H], F32, tag="sg", name="sg")
                        sc.activation(sg[:], zb[:, H:], AF.Sigmoid)
                        if l == 0:
                            ub = ubp.tile([128, H], F32, tag="ub", name="ub")
                            nc.sync.dma_start(ub[:], u0b_in[b2, ct, :, :])
                            zt = zwp.tile([128, H], F32, tag="zt", name="zt")
                            v.tensor_mul(zt[:], zb[:, :H], sg[:])
                            v.tensor_add(zs3[b2][:, :, ct], zt[:], ub[:])
                        else:
                            osb = zwp.tile([128, H], F32, tag="osb", name="osb")
                            v.tensor_mul(osb[:], zb[:, :H], sg[:])
                            nc.sync.dma_start(out_z[b2, ct * T : (ct + 1) * T, :], osb[:])
                if l == 0:
                    for d in range(CORES):
                        for b2 in range(B2):
                            dst = a2a_u_in[d, :, b2, :, :].rearrange("j hh c -> j (hh c)")
                            nc.sync.dma_start(
                                dst, zstage[b2][:, d * HS * C : (d + 1) * HS * C])
                    gp.collective_compute(
                        "AllToAll", OP.bypass, replica_groups=RG,
                        ins=[a2a_u_in[:].opt()], outs=[a2a_u_out[:].opt()])
                    uv = u_sb[:].rearrange("j (h b c) -> j h b c", b=B, c=C)
                    for s in range(CORES):
                        for b2 in range(B2):
                            nc.sync.dma_start(
                                uv[:, :, 2 * s + b2, :],
                                a2a_u_out[s, :, b2, :, :])
    nc.finalize()
    _NC_CACHE[key] = nc
    return nc


# ====================== host side ======================

def _prep_core_inputs(core, x, pars):
    hs = slice(HS * core, HS * (core + 1))
    ins = {}
    xs = x[:, :, hs]                                    # (B, L, 64)
    u0 = xs.reshape(B, C, T, HS).transpose(2, 3, 0, 1)  # (j, h, b, c)
    ins["u0"] = np.ascontiguousarray(u0)
    xb = x[B2 * core : B2 * (core + 1)]                 # (2, L, H)
    ins["u0b"] = np.ascontiguousarray(xb.reshape(B2, C, T, H))
    ins["trimask"] = np.triu(np.ones((T, T), np.float32))
    ins["ident"] = np.eye(T, dtype=np.float32)

    def scan_layout(a):
        if a.ndim == 1:
            a = np.broadcast_to(a[:, None], (HS, N))
        return np.ascontiguousarray(
            a.reshape(NHB, 2, N).transpose(1, 2, 0).reshape(128, NHB))

    for l in (0, 1):
        ins[f"ldt{l}"] = scan_layout(pars[f"ldt{l}"][hs])
        ins[f"lare{l}"] = scan_layout(pars[f"lAre{l}"][hs])
        ins[f"aim{l}"] = scan_layout(pars[f"Aim{l}"][hs])
        ins[f"cre{l}"] = scan_layout(pars[f"Cre{l}"][hs])
        ins[f"cim{l}"] = scan_layout(pars[f"Cim{l}"][hs])
        ins[f"drep{l}"] = np.ascontiguousarray(
            np.broadcast_to(pars[f"D{l}"][hs][None, :], (128, HS)))
        ins[f"wt{l}"] = np.ascontiguousarray(pars[f"W{l}"].T)
        ins[f"brow{l}"] = np.ascontiguousarray(pars[f"b{l}"][None, :])
    out = {k: vv.astype(np.float32) for k, vv in ins.items()}
    out["u0"] = ins["u0"].astype(ml_dtypes.bfloat16)
    for l in (0, 1):
        out[f"wt{l}"] = ins[f"wt{l}"].astype(ml_dtypes.bfloat16)
    return out


def run(x, pars, debug=False, trace=False):
    nc = build_kernel(debug=debug)
    in_maps = [_prep_core_inputs(c, x, pars) for c in range(CORES)]
    r = run_bass_kernel_spmd(nc, in_maps, core_ids=list(range(CORES)), trace=trace)
    outs = np.stack([r.results[c]["out"] for c in range(CORES)])  # (8, 2, L, H)
    full = outs.reshape(B, L, H)
    return full, r


def kernel(**inputs):
    x = np.asarray(inputs["x"], dtype=np.float32)
    pars = {k: np.asarray(vv, dtype=np.float32) for k, vv in inputs.items() if k != "x"}
    full, _ = run(x, pars)
    return full


# revision 36
# speedup vs baseline: 1.8995x; 1.0292x over previous
"""Trainium2 Bass kernel for the 2-layer S4D block (nn_MetaS4History).

Strategy (8 cores, single launch):
  - Conv phases H-sharded (64 channels/core, full batch): chunked-SSD convolution
    with T=128 chunks: per-h matmuls (G-build, intra, injection) + a 16-step
    DVE scan for inter-chunk states. Matmuls in bf16 (PSUM fp32 accumulate).
  - GLU projections B-sharded (2 batch elems/core, full H): y-stationary bf16
    matmuls producing [l-pos, 2H] tiles (no output transposes needed).
  - Phase boundaries resharded with AllToAll collectives (3 total, bf16).
Layouts: u_sb [j,(h,b,c)] (contiguous matmul operands); X states
[128,(c,hb,b)] (contiguous scan); zstage [j,(h,c)] (contiguous a2a DMAs).
"""
import contextlib
import ml_dtypes
import numpy as np
import concourse.bacc as bacc
import concourse.mybir as mybir
from concourse.tile import TileContext
from concourse.bass_utils import run_bass_kernel_spmd

F32 = mybir.dt.float32
BF16 = mybir.dt.bfloat16
AF = mybir.ActivationFunctionType
OP = mybir.AluOpType

CORES = 8
B, L, H, N = 16, 2048, 512, 64
T, C = 128, 16          # chunk len, chunk count
HS = H // CORES         # 64 channels per core
B2 = B // CORES         # 2 batch per core (GLU phase)
NHB = HS // 2           # 32 h-blocks (h = 2*hblk + hpar)
RG = [list(range(CORES))]

_NC_CACHE = {}


def _build_table(eng, tre, tim, seed_re, seed_im, mul_re, mul_im, wk, nhb):
    """Power table via doubling: tab[:, hb, j] = seed * mul^j, j in 0..T-1.
    tre/tim: [128, nhb*T] tiles; seed/mul: [128, nhb] APs (sliced); wk: pool."""
    t3re = tre[:].rearrange("p (h j) -> p h j", j=T)
    t3im = tim[:].rearrange("p (h j) -> p h j", j=T)
    eng.tensor_copy(t3re[:, :, 0:1], seed_re.unsqueeze(2))
    eng.tensor_copy(t3im[:, :, 0:1], seed_im.unsqueeze(2))
    mre = wk.tile([128, nhb], F32, tag="dbl_mre", name="dbl_mre")
    mim = wk.tile([128, nhb], F32, tag="dbl_mim", name="dbl_mim")
    q1 = wk.tile([128, nhb], F32, tag="dbl_q1", name="dbl_q1")
    q2 = wk.tile([128, nhb], F32, tag="dbl_q2", name="dbl_q2")
    sc1 = wk.tile([128, nhb * T // 2], F32, tag="dbl_s1", name="dbl_s1")
    eng.tensor_copy(mre[:], mul_re)
    eng.tensor_copy(mim[:], mul_im)
    m = 1
    while m < T:
        mbre = mre[:].unsqueeze(2).broadcast_to([128, nhb, m])
        mbim = mim[:].unsqueeze(2).broadcast_to([128, nhb, m])
        s1 = sc1[:].rearrange("p (h j) -> p h j", j=T // 2)[:, :, 0:m]
        src_re, src_im = t3re[:, :, 0:m], t3im[:, :, 0:m]
        dst_re, dst_im = t3re[:, :, m : 2 * m], t3im[:, :, m : 2 * m]
        eng.tensor_mul(s1, src_im, mbim)
        eng.tensor_mul(dst_re, src_re, mbre)
        eng.tensor_sub(dst_re, dst_re, s1)
        eng.tensor_mul(s1, src_im, mbre)
        eng.tensor_mul(dst_im, src_re, mbim)
        eng.tensor_add(dst_im, dst_im, s1)
        m *= 2
        if m < T:
            eng.tensor_mul(q1[:], mre[:], mre[:])
            eng.tensor_mul(q2[:], mim[:], mim[:])
            eng.tensor_mul(mim[:], mre[:], mim[:])
            eng.tensor_add(mim[:], mim[:], mim[:])
            eng.tensor_sub(mre[:], q1[:], q2[:])


def build_kernel(debug=False):
    key = debug
    if key in _NC_CACHE:
        return _NC_CACHE[key]
    nc = bacc.Bacc(num_devices=CORES)
    v = nc.vector
    gp = nc.gpsimd
    sc = nc.scalar
    te = nc.tensor

    # ---------------- DRAM I/O ----------------
    u0_in = nc.dram_tensor("u0", [T, HS, B, C], BF16, kind="ExternalInput")
    u0b_in = nc.dram_tensor("u0b", [B2, C, T, H], F32, kind="ExternalInput")
    trimask_in = nc.dram_tensor("trimask", [T, T], F32, kind="ExternalInput")
    ident_in = nc.dram_tensor("ident", [T, T], F32, kind="ExternalInput")
    par_in = {}
    for l in (0, 1):
        for nm in ("ldt", "lare", "aim", "cre", "cim"):
            par_in[(nm, l)] = nc.dram_tensor(f"{nm}{l}", [128, NHB], F32, kind="ExternalInput")
        par_in[("drep", l)] = nc.dram_tensor(f"drep{l}", [128, HS], F32, kind="ExternalInput")
        par_in[("wt", l)] = nc.dram_tensor(f"wt{l}", [H, 2 * H], BF16, kind="ExternalInput")
        par_in[("brow", l)] = nc.dram_tensor(f"brow{l}", [1, 2 * H], F32, kind="ExternalInput")

    HH = HS // 2   # h-half for split y collectives
    a2a_y_in = [[nc.dram_tensor(f"a2aY{l}{p}_in", [CORES, HH, B2, L], BF16) for p in (0, 1)]
                for l in (0, 1)]
    a2a_y_out = [[nc.dram_tensor(f"a2aY{l}{p}_out", [CORES, HH, B2, L], BF16) for p in (0, 1)]
                 for l in (0, 1)]
    a2a_u_in = [nc.dram_tensor(f"a2aU{b2}_in", [CORES, T, HS, C], BF16) for b2 in range(B2)]
    a2a_u_out = [nc.dram_tensor(f"a2aU{b2}_out", [CORES, T, HS, C], BF16) for b2 in range(B2)]
    out_z = nc.dram_tensor("out", [B2, L, H], F32, kind="ExternalOutput")

    with TileContext(nc) as tc, contextlib.ExitStack() as top:
        cpool = top.enter_context(tc.tile_pool(name="consts", bufs=1))
        trimask = cpool.tile([T, T], F32, tag="trimask", name="trimask")
        ident = cpool.tile([T, T], F32, tag="ident", name="ident")
        nc.sync.dma_start(trimask[:], trimask_in[:])
        nc.sync.dma_start(ident[:], ident_in[:])
        csts = cpool.tile([128, 32], F32, tag="csts", name="csts")
        SINC = [1.0, -1.0 / 6, 1.0 / 120, -1.0 / 5040, 1.0 / 362880, -1.0 / 39916800]
        COSC = [1.0, -1.0 / 2, 1.0 / 24, -1.0 / 720, 1.0 / 40320, -1.0 / 3628800]
        for k in range(6):
            nc.any.memset(csts[:, k : k + 1], SINC[k])
            nc.any.memset(csts[:, 6 + k : 7 + k], COSC[k])
        nc.any.memset(csts[:, 12:13], -1.0)
        nc.any.memset(csts[:, 13:14], 2.0)
        nc.any.memset(csts[:, 14:15], 1.0 / 16)
        import math
        for k in range(11):
            nc.any.memset(csts[:, 16 + k : 17 + k], 1.0 / math.factorial(k))
        nc.any.memset(csts[:, 27:28], 1.0 / 8)

        upool = top.enter_context(tc.tile_pool(name="u", bufs=1))
        u_sb = upool.tile([T, HS * B * C], BF16, tag="u_sb", name="u_sb")  # [j,(h,b,c)]
        nc.sync.dma_start(u_sb[:], u0_in[:].rearrange("j h b c -> j (h b c)"))

        def u_slice(h, bq=None):
            b0, nb = (0, B) if bq is None else (bq * 8, 8)
            ap = u_sb[:].rearrange("j (h b c) -> j h b c", b=B, c=C)
            return ap[:, h, b0 : b0 + nb, :]

        for l in (0, 1):
            # ======== CONV PHASE (H-shard) ========
            with contextlib.ExitStack() as cv:
                pp = cv.enter_context(tc.tile_pool(name=f"par{l}", bufs=1))
                P = {}
                for nm in ("ldt", "lare", "aim", "cre", "cim"):
                    P[nm] = pp.tile([128, NHB], F32, tag=f"p_{nm}", name=f"p_{nm}")
                    nc.sync.dma_start(P[nm][:], par_in[(nm, l)][:])
                drep = pp.tile([128, HS], F32, tag="p_drep", name="p_drep")
                nc.sync.dma_start(drep[:], par_in[("drep", l)][:])

                def wk(tag):
                    return pp.tile([128, NHB], F32, tag=tag, name=tag)[:]

                neg1 = csts[:, 12:13]
                two = csts[:, 13:14]
                s16 = csts[:, 14:15]

                def exp_poly(out, x):
                    """out = e^x via (T10(x/8))^8; |x| <= 8. Accurate to ~1e-7."""
                    ea = wk("exp_a")
                    et = wk("exp_t")
                    v.tensor_scalar(ea, x, csts[:, 27:28], None, op0=OP.mult)  # y = x/8
                    v.tensor_scalar(et, ea, csts[:, 26:27], csts[:, 25:26], op0=OP.mult, op1=OP.add)
                    for k in range(8, -1, -1):
                        v.tensor_mul(et, et, ea)
                        v.tensor_scalar(et, et, csts[:, 16 + k : 17 + k], None, op0=OP.add)
                    for _ in range(3):
                        v.tensor_mul(et, et, et)
                    v.tensor_copy(out, et)

                dt, eA = wk("dt"), wk("eA")
                exp_poly(dt, P["ldt"][:])
                exp_poly(eA, P["lare"][:])
                dtAre, dtAim = wk("dtAre"), wk("dtAim")
                v.scalar_tensor_tensor(dtAre, dt, -1.0, eA, op0=OP.mult, op1=OP.mult)
                v.tensor_mul(dtAim, dt, P["aim"][:])
                mag = wk("mag")
                exp_poly(mag, dtAre)
                q, x2 = wk("q"), wk("x2")
                v.tensor_scalar(q, dtAim, s16, None, op0=OP.mult)
                v.tensor_mul(x2, q, q)
                acc, t1, t2 = wk("acc"), wk("t1"), wk("t2")
                cr, ci = wk("cr"), wk("ci")
                v.tensor_scalar(acc, x2, csts[:, 5:6], csts[:, 4:5], op0=OP.mult, op1=OP.add)
                for k in (3, 2, 1, 0):
                    v.tensor_mul(t1, acc, x2)
                    v.tensor_scalar(acc, t1, csts[:, k : k + 1], None, op0=OP.add)
                v.tensor_mul(ci, acc, q)
                v.tensor_scalar(acc, x2, csts[:, 11:12], csts[:, 10:11], op0=OP.mult, op1=OP.add)
                for k in (9, 8, 7, 6):
                    v.tensor_mul(t1, acc, x2)
                    v.tensor_scalar(acc, t1, csts[:, k : k + 1], None, op0=OP.add)
                v.tensor_copy(cr, acc)
                for _ in range(4):
                    v.tensor_mul(t1, cr, cr)
                    v.tensor_mul(t2, ci, ci)
                    v.scalar_tensor_tensor(acc, cr, 2.0, ci, op0=OP.mult, op1=OP.mult)
                    v.tensor_sub(cr, t1, t2)
                    v.tensor_copy(ci, acc)
                wre, wim = wk("wre"), wk("wim")
                v.tensor_mul(wre, mag, cr)
                v.tensor_mul(wim, mag, ci)
                m2, im2 = wk("m2"), wk("im2")
                v.tensor_mul(m2, mag, mag)
                v.reciprocal(im2, m2)
                rpre, rpim = wk("rpre"), wk("rpim")
                v.tensor_mul(rpre, wre, im2)
                v.tensor_mul(rpim, wim, im2)
                wm1re = wk("wm1re")
                v.tensor_scalar(wm1re, wre, neg1, None, op0=OP.add)
                tre, tim = wk("tre"), wk("tim")
                v.tensor_mul(t1, P["cre"][:], wm1re)
                v.tensor_mul(t2, P["cim"][:], wim)
                v.tensor_sub(tre, t1, t2)
                v.tensor_mul(t1, P["cre"][:], wim)
                v.tensor_mul(t2, P["cim"][:], wm1re)
                v.tensor_add(tim, t1, t2)
                den, invd = wk("den"), wk("invd")
                v.tensor_mul(t1, eA, eA)
                v.tensor_mul(t2, P["aim"][:], P["aim"][:])
                v.tensor_add(den, t1, t2)
                v.reciprocal(invd, den)
                ccr, cci = wk("ccr"), wk("cci")
                v.tensor_mul(t1, tre, eA)
                v.tensor_mul(t2, tim, P["aim"][:])
                v.tensor_sub(acc, t2, t1)
                v.tensor_mul(ccr, acc, invd)
                v.tensor_mul(t1, tre, P["aim"][:])
                v.tensor_mul(t2, tim, eA)
                v.tensor_add(acc, t1, t2)
                v.tensor_mul(t1, acc, invd)
                v.tensor_scalar(cci, t1, neg1, None, op0=OP.mult)
                esr, esi = wk("esr"), wk("esi")
                v.tensor_mul(t1, ccr, wre)
                v.tensor_mul(t2, cci, wim)
                v.tensor_sub(acc, t1, t2)
                v.tensor_scalar(esr, acc, two, None, op0=OP.mult)
                v.tensor_mul(t1, ccr, wim)
                v.tensor_mul(t2, cci, wre)
                v.tensor_add(acc, t1, t2)
                v.tensor_scalar(esi, acc, two, None, op0=OP.mult)
                wtr, wti = wk("wtr"), wk("wti")
                v.tensor_copy(wtr, wre)
                v.tensor_copy(wti, wim)
                for _ in range(7):
                    v.tensor_mul(t1, wtr, wtr)
                    v.tensor_mul(t2, wti, wti)
                    v.scalar_tensor_tensor(acc, wtr, 2.0, wti, op0=OP.mult, op1=OP.mult)
                    v.tensor_sub(wtr, t1, t2)
                    v.tensor_copy(wti, acc)
                dre, dim_ = wk("dre"), wk("dim")
                v.tensor_copy(dre, wtr)
                v.tensor_scalar(dim_, wti, neg1, None, op0=OP.mult)

                # ---------- full-width tables ----------
                tpr = cv.enter_context(tc.tile_pool(name=f"tabR{l}", bufs=1))
                Rp_re = tpr.tile([128, NHB * T], F32, tag="Rp_re", name="Rp_re")
                Rp_im = tpr.tile([128, NHB * T], F32, tag="Rp_im", name="Rp_im")
                R16re = tpr.tile([128, NHB * T], BF16, tag="R16re", name="R16re")
                R16im = tpr.tile([128, NHB * T], BF16, tag="R16im", name="R16im")
                with tc.tile_pool(name=f"dblR{l}", bufs=1) as dwk:
                    _build_table(v, Rp_re, Rp_im, rpre[:], rpim[:],
                                 rpre[:], rpim[:], dwk, NHB)
                sc.activation(R16re[:], Rp_re[:], AF.Copy)
                v.tensor_copy(R16im[:], Rp_im[:])

                tpe = cv.enter_context(tc.tile_pool(name=f"tabE{l}", bufs=1))
                E_re = tpe.tile([128, NHB * T], F32, tag="E_re", name="E_re")
                E_im = tpe.tile([128, NHB * T], F32, tag="E_im", name="E_im")
                E16re = tpe.tile([128, NHB * T], BF16, tag="E16re", name="E16re")
                E16im = tpe.tile([128, NHB * T], BF16, tag="E16im", name="E16im")
                with tc.tile_pool(name=f"dblE{l}", bufs=1) as dwk:
                    _build_table(gp, E_re, E_im, esr[:], esi[:],
                                 wre[:], wim[:], dwk, NHB)
                sc.activation(E16re[:], E_re[:], AF.Copy)
                gp.tensor_copy(E16im[:], E_im[:])

                # ---------- collection: X[p, (hb, b, c)] chunk states ----------
                stp = cv.enter_context(tc.tile_pool(name=f"st{l}", bufs=1))
                X_re = stp.tile([128, NHB * B * C], BF16, tag="X_re", name="X_re")
                X_im = stp.tile([128, NHB * B * C], BF16, tag="X_im", name="X_im")
                X_re4 = X_re[:].rearrange("p (h b c) -> p h b c", b=B, c=C)
                X_im4 = X_im[:].rearrange("p (h b c) -> p h b c", b=B, c=C)

                with tc.tile_pool(name=f"wsl{l}", bufs=3) as wslp, \
                     tc.tile_pool(name=f"pst{l}", bufs=2, space="PSUM") as pstp, \
                     tc.tile_pool(name=f"psc{l}", bufs=2, space="PSUM") as pscp:
                    for k in range(NHB):
                        wsl = [wslp.tile([128, T], BF16, tag=f"wsl{comp}", name=f"wsl{comp}")
                               for comp in (0, 1)]
                        for comp, Rt in enumerate((Rp_re, Rp_im)):
                            psT = pstp.tile([128, T], F32, tag="psT", name="psT")
                            te.transpose(psT[:], Rt[:, k * T : (k + 1) * T], ident[:])
                            sc.activation(wsl[comp][:], psT[:], AF.Copy)
                        psr = pscp.tile([128, B * C], F32, tag="psr", name="psr")
                        psi = pscp.tile([128, B * C], F32, tag="psi", name="psi")
                        for hp in (0, 1):
                            h = 2 * k + hp
                            us = u_slice(h)
                            te.matmul(psr[64 * hp : 64 * hp + 64, :],
                                      wsl[0][:, 64 * hp : 64 * hp + 64], us, start=True, stop=True)
                            te.matmul(psi[64 * hp : 64 * hp + 64, :],
                                      wsl[1][:, 64 * hp : 64 * hp + 64], us, start=True, stop=True)
                        sc.activation(X_re4[:, k, :, :], psr[:], AF.Copy)
                        sc.activation(X_im4[:, k, :, :], psi[:], AF.Copy)

                # ---------- scan (in place: X becomes S_before) ----------
                # All-vector chain (in-order, no cross-engine sync); state
                # ping-pongs between parities so the X snapshots (sc/gp) are
                # off the critical path. bf16 temporaries halve DVE time.
                with tc.tile_pool(name=f"scan{l}", bufs=1) as sp:
                    def stile(nm, dt_=F32):
                        return sp.tile([128, NHB * B], dt_, tag=nm, name=nm)[:].rearrange(
                            "p (h b) -> p h b", b=B)
                    Spp = [[stile("Sr0"), stile("Sr1")], [stile("Si0"), stile("Si1")]]
                    t_r3, t_i3 = stile("tm_r", BF16), stile("tm_i", BF16)
                    w1r, w2r = stile("w1r", BF16), stile("w2r", BF16)
                    w1i, w2i = stile("w1i", BF16), stile("w2i", BF16)
                    nc.any.memset(Spp[0][0], 0.0)
                    nc.any.memset(Spp[1][0], 0.0)
                    dreb = dre[:].unsqueeze(2).broadcast_to([128, NHB, B])
                    dimb = dim_[:].unsqueeze(2).broadcast_to([128, NHB, B])
                    for ccc in range(C):
                        pr, nx = ccc % 2, (ccc + 1) % 2
                        Sr3, Si3 = Spp[0][pr], Spp[1][pr]
                        Srn, Sin = Spp[0][nx], Spp[1][nx]
                        xr, xi = X_re4[:, :, :, ccc], X_im4[:, :, :, ccc]
                        v.tensor_add(t_r3, Sr3, xr)
                        v.tensor_add(t_i3, Si3, xi)
                        sc.activation(xr, Sr3, AF.Copy)
                        gp.tensor_copy(xi, Si3)
                        v.tensor_mul(w1r, t_r3, dreb)
                        v.tensor_mul(w2r, t_i3, dimb)
                        v.tensor_sub(Srn, w1r, w2r)
                        v.tensor_mul(w1i, t_i3, dreb)
                        v.tensor_mul(w2i, t_r3, dimb)
                        v.tensor_add(Sin, w1i, w2i)

                # ---------- per-h conv ----------
                gt_pool = cv.enter_context(tc.tile_pool(name=f"gt{l}", bufs=3))
                ya_pool = cv.enter_context(tc.tile_pool(name=f"ya{l}", bufs=2))
                gp_ps = cv.enter_context(tc.tile_pool(name=f"gps{l}", bufs=2, space="PSUM"))
                cv_ps = cv.enter_context(tc.tile_pool(name=f"cvps{l}", bufs=2, space="PSUM"))
                HG = 8
                yg = [None, None]
                for h in range(HS):
                    hp, hb = h & 1, h >> 1
                    base = 64 * hp
                    er = E16re[base : base + 64, hb * T : (hb + 1) * T]
                    ei = E16im[base : base + 64, hb * T : (hb + 1) * T]
                    rr = R16re[base : base + 64, hb * T : (hb + 1) * T]
                    ri = R16im[base : base + 64, hb * T : (hb + 1) * T]
                    psG = gp_ps.tile([128, T], F32, tag="psG", name="psG")
                    te.matmul(psG[:], rr, er, start=True, stop=False)
                    te.matmul(psG[:], ri, ei, start=False, stop=True)
                    GTt = gt_pool.tile([128, T], F32, tag="GTt", name="GTt")
                    GT = gt_pool.tile([128, T], BF16, tag="GT", name="GT")
                    v.tensor_mul(GTt[:], psG[:], trimask[:])
                    v.scalar_tensor_tensor(GT[:], ident[:], drep[:, h : h + 1], GTt[:],
                                           op0=OP.mult, op1=OP.add)
                    if h % HG == 0:
                        yg = [ya_pool.tile([128, HG * T], BF16, tag=f"yg{qq}", name=f"yg{qq}")
                              for qq in (0, 1)]
                    for qq in (0, 1):
                        ps = cv_ps.tile([128, T], F32, tag="ps", name="ps")
                        lu = u_slice(h, qq)
                        te.matmul(ps[:], lu, GT[:], start=True, stop=False)
                        lr = X_re[base : base + 64,
                                  hb * B * C + qq * 128 : hb * B * C + qq * 128 + 128]
                        li = X_im[base : base + 64,
                                  hb * B * C + qq * 128 : hb * B * C + qq * 128 + 128]
                        te.matmul(ps[:], lr, er, start=False, stop=False)
                        te.matmul(ps[:], li, ei, start=False, stop=True)
                        sc.activation(yg[qq][:, (h % HG) * T : (h % HG + 1) * T], ps[:],
                                      AF.Gelu_apprx_tanh)
                    if h % HG == HG - 1:
                        hg0 = h - HG + 1
                        p = hg0 // HH
                        hg0p = hg0 % HH
                        for qq in (0, 1):
                            ygv = yg[qq][:].rearrange("bc (hh2 j) -> bc hh2 j", j=T)
                            for dd in range(4):
                                d = qq * 4 + dd
                                dst = a2a_y_in[l][p][d, hg0p : hg0p + HG, :, :].rearrange(
                                    "hh2 b2 (c j) -> (b2 c) hh2 j", j=T)
                                nc.sync.dma_start(dst, ygv[32 * dd : 32 * dd + 32, :, :])
                        if h == HH - 1 or h == HS - 1:
                            gp.collective_compute(
                                "AllToAll", OP.bypass, replica_groups=RG,
                                ins=[a2a_y_in[l][p][:].opt()],
                                outs=[a2a_y_out[l][p][:].opt()])

            # ======== GLU PHASE (B-shard) ========
            with contextlib.ExitStack() as gl:
                gpool = gl.enter_context(tc.tile_pool(name=f"glu{l}", bufs=1))
                wtiles = [gpool.tile([128, 2 * H], BF16, tag=f"wt{k}", name=f"wt{k}") for k in range(4)]
                ytiles = [gpool.tile([128, B2 * L], BF16, tag=f"yk{k}", name=f"yk{k}") for k in range(4)]
                for kt in range(4):
                    nc.sync.dma_start(wtiles[kt][:], par_in[("wt", l)][128 * kt : 128 * (kt + 1), :])
                    for si in (0, 1):
                        s = 2 * kt + si
                        for p in (0, 1):
                            nc.sync.dma_start(
                                ytiles[kt][64 * si + HH * p : 64 * si + HH * (p + 1), :],
                                a2a_y_out[l][p][s].rearrange("h b2 ll -> h (b2 ll)"))
                bb = gpool.tile([128, 2 * H], F32, tag="bb", name="bb")
                nc.sync.dma_start(bb[:], par_in[("brow", l)][:].broadcast_to([128, 2 * H]))
                if l == 0:
                    zstage = [gpool.tile([128, H * C], BF16, tag=f"zst{b2}", name=f"zst{b2}")
                              for b2 in range(B2)]
                    zs3 = [z[:].rearrange("p (h c) -> p h c", c=C) for z in zstage]
                zps = gl.enter_context(tc.tile_pool(name=f"zps{l}", bufs=2, space="PSUM"))
                zwp = gl.enter_context(tc.tile_pool(name=f"zw{l}", bufs=3))
                ubp = gl.enter_context(tc.tile_pool(name=f"ub{l}", bufs=2))
                for b2 in range(B2):
                    if l == 0:
                        ub3 = ubp.tile([128, C * H], F32, tag="ub3", name="ub3")[:].rearrange(
                            "p (c h) -> p c h", c=C)
                        nc.sync.dma_start(ub3, u0b_in[b2].rearrange("c j h -> j c h"))
                    for ct in range(C):
                        psZ1 = zps.tile([128, H], F32, tag="psZ1", name="psZ1")
                        psZ2 = zps.tile([128, H], F32, tag="psZ2", name="psZ2")
                        for kt in range(4):
                            yst = ytiles[kt][:, b2 * L + ct * T : b2 * L + (ct + 1) * T]
                            te.matmul(psZ1[:], yst, wtiles[kt][:, :H],
                                      start=(kt == 0), stop=(kt == 3))
                            te.matmul(psZ2[:], yst, wtiles[kt][:, H:],
                                      start=(kt == 0), stop=(kt == 3))
                        zb = zwp.tile([128, 2 * H], F32, tag="zb", name="zb")
                        v.tensor_add(zb[:, :H], psZ1[:], bb[:, :H])
                        v.tensor_add(zb[:, H:], psZ2[:], bb[:, H:])
                        sg = zwp.tile([128, H], F32, tag="sg", name="sg")
                        sc.activation(sg[:], zb[:, H:], AF.Sigmoid)
                        if l == 0:
                            zt = zwp.tile([128, H], F32, tag="zt", name="zt")
                            v.tensor_mul(zt[:], zb[:, :H], sg[:])
                            v.tensor_add(zs3[b2][:, :, ct], zt[:], ub3[:, ct, :])
                        else:
                            osb = zwp.tile([128, H], F32, tag="osb", name="osb")
                            v.tensor_mul(osb[:], zb[:, :H], sg[:])
                            nc.sync.dma_start(out_z[b2, ct * T : (ct + 1) * T, :], osb[:])
                    if l == 0:
                        for d in range(CORES):
                            dst = a2a_u_in[b2][d].rearrange("j hh c -> j (hh c)")
                            nc.sync.dma_start(
                                dst, zstage[b2][:, d * HS * C : (d + 1) * HS * C])
                        gp.collective_compute(
                            "AllToAll", OP.bypass, replica_groups=RG,
                            ins=[a2a_u_in[b2][:].opt()], outs=[a2a_u_out[b2][:].opt()])
                if l == 0:
                    uv = u_sb[:].rearrange("j (h b c) -> j h b c", b=B, c=C)
                    for s in range(CORES):
                        for b2 in range(B2):
                            nc.sync.dma_start(
                                uv[:, :, 2 * s + b2, :],
                                a2a_u_out[b2][s])
    nc.finalize()
    _NC_CACHE[key] = nc
    return nc


# ====================== host side ======================

def _prep_core_inputs(core, x, pars):
    hs = slice(HS * core, HS * (core + 1))
    ins = {}
    xs = x[:, :, hs]                                    # (B, L, 64)
    u0 = xs.reshape(B, C, T, HS).transpose(2, 3, 0, 1)  # (j, h, b, c)
    ins["u0"] = np.ascontiguousarray(u0)
    xb = x[B2 * core : B2 * (core + 1)]                 # (2, L, H)
    ins["u0b"] = np.ascontiguousarray(xb.reshape(B2, C, T, H))
    ins["trimask"] = np.triu(np.ones((T, T), np.float32))
    ins["ident"] = np.eye(T, dtype=np.float32)

    def scan_layout(a):
        if a.ndim == 1:
            a = np.broadcast_to(a[:, None], (HS, N))
        return np.ascontiguousarray(
            a.reshape(NHB, 2, N).transpose(1, 2, 0).reshape(128, NHB))

    for l in (0, 1):
        ins[f"ldt{l}"] = scan_layout(pars[f"ldt{l}"][hs])
        ins[f"lare{l}"] = scan_layout(pars[f"lAre{l}"][hs])
        ins[f"aim{l}"] = scan_layout(pars[f"Aim{l}"][hs])
        ins[f"cre{l}"] = scan_layout(pars[f"Cre{l}"][hs])
        ins[f"cim{l}"] = scan_layout(pars[f"Cim{l}"][hs])
        ins[f"drep{l}"] = np.ascontiguousarray(
            np.broadcast_to(pars[f"D{l}"][hs][None, :], (128, HS)))
        ins[f"wt{l}"] = np.ascontiguousarray(pars[f"W{l}"].T)
        ins[f"brow{l}"] = np.ascontiguousarray(pars[f"b{l}"][None, :])
    out = {k: vv.astype(np.float32) for k, vv in ins.items()}
    out["u0"] = ins["u0"].astype(ml_dtypes.bfloat16)
    for l in (0, 1):
        out[f"wt{l}"] = ins[f"wt{l}"].astype(ml_dtypes.bfloat16)
    return out


def run(x, pars, debug=False, trace=False):
    nc = build_kernel(debug=debug)
    in_maps = [_prep_core_inputs(c, x, pars) for c in range(CORES)]
    r = run_bass_kernel_spmd(nc, in_maps, core_ids=list(range(CORES)), trace=trace)
    outs = np.stack([r.results[c]["out"] for c in range(CORES)])  # (8, 2, L, H)
    full = outs.reshape(B, L, H)
    return full, r


def kernel(**inputs):
    x = np.asarray(inputs["x"], dtype=np.float32)
    pars = {k: np.asarray(vv, dtype=np.float32) for k, vv in inputs.items() if k != "x"}
    full, _ = run(x, pars)
    return full


# revision 42
# speedup vs baseline: 2.0000x; 1.0529x over previous
"""Trainium2 Bass kernel for the 2-layer S4D block (nn_MetaS4History).

Strategy (8 cores, single launch):
  - Conv phases H-sharded (64 channels/core, full batch): chunked-SSD convolution
    with T=128 chunks: per-h matmuls (G-build, intra, injection) + a 16-step
    DVE scan for inter-chunk states. Matmuls in bf16 (PSUM fp32 accumulate).
  - GLU projections B-sharded (2 batch elems/core, full H): y-stationary bf16
    matmuls producing [l-pos, 2H] tiles (no output transposes needed).
  - Phase boundaries resharded with AllToAll collectives (3 total, bf16).
Layouts: u_sb [j,(h,b,c)] (contiguous matmul operands); X states
[128,(c,hb,b)] (contiguous scan); zstage [j,(h,c)] (contiguous a2a DMAs).
"""
import contextlib
import ml_dtypes
import numpy as np
import concourse.bacc as bacc
import concourse.mybir as mybir
from concourse.tile import TileContext
from concourse.bass_utils import run_bass_kernel_spmd

F32 = mybir.dt.float32
BF16 = mybir.dt.bfloat16
AF = mybir.ActivationFunctionType
OP = mybir.AluOpType

CORES = 8
B, L, H, N = 16, 2048, 512, 64
T, C = 128, 16          # chunk len, chunk count
HS = H // CORES         # 64 channels per core
B2 = B // CORES         # 2 batch per core (GLU phase)
NHB = HS // 2           # 32 h-blocks (h = 2*hblk + hpar)
RG = [list(range(CORES))]

_NC_CACHE = {}


def _build_table(eng, tre, tim, seed_re, seed_im, mul_re, mul_im, wk, nhb):
    """Power table via doubling: tab[:, hb, j] = seed * mul^j, j in 0..T-1.
    tre/tim: [128, nhb*T] tiles; seed/mul: [128, nhb] APs (sliced); wk: pool."""
    t3re = tre[:].rearrange("p (h j) -> p h j", j=T)
    t3im = tim[:].rearrange("p (h j) -> p h j", j=T)
    eng.tensor_copy(t3re[:, :, 0:1], seed_re.unsqueeze(2))
    eng.tensor_copy(t3im[:, :, 0:1], seed_im.unsqueeze(2))
    mre = wk.tile([128, nhb], F32, tag="dbl_mre", name="dbl_mre")
    mim = wk.tile([128, nhb], F32, tag="dbl_mim", name="dbl_mim")
    q1 = wk.tile([128, nhb], F32, tag="dbl_q1", name="dbl_q1")
    q2 = wk.tile([128, nhb], F32, tag="dbl_q2", name="dbl_q2")
    sc1 = wk.tile([128, nhb * T // 2], F32, tag="dbl_s1", name="dbl_s1")
    eng.tensor_copy(mre[:], mul_re)
    eng.tensor_copy(mim[:], mul_im)
    m = 1
    while m < T:
        mbre = mre[:].unsqueeze(2).broadcast_to([128, nhb, m])
        mbim = mim[:].unsqueeze(2).broadcast_to([128, nhb, m])
        s1 = sc1[:].rearrange("p (h j) -> p h j", j=T // 2)[:, :, 0:m]
        src_re, src_im = t3re[:, :, 0:m], t3im[:, :, 0:m]
        dst_re, dst_im = t3re[:, :, m : 2 * m], t3im[:, :, m : 2 * m]
        eng.tensor_mul(s1, src_im, mbim)
        eng.tensor_mul(dst_re, src_re, mbre)
        eng.tensor_sub(dst_re, dst_re, s1)
        eng.tensor_mul(s1, src_im, mbre)
        eng.tensor_mul(dst_im, src_re, mbim)
        eng.tensor_add(dst_im, dst_im, s1)
        m *= 2
        if m < T:
            eng.tensor_mul(q1[:], mre[:], mre[:])
            eng.tensor_mul(q2[:], mim[:], mim[:])
            eng.tensor_mul(mim[:], mre[:], mim[:])
            eng.tensor_add(mim[:], mim[:], mim[:])
            eng.tensor_sub(mre[:], q1[:], q2[:])


def build_kernel(debug=False):
    key = debug
    if key in _NC_CACHE:
        return _NC_CACHE[key]
    nc = bacc.Bacc(num_devices=CORES)
    v = nc.vector
    gp = nc.gpsimd
    sc = nc.scalar
    te = nc.tensor

    # ---------------- DRAM I/O ----------------
    u0_in = nc.dram_tensor("u0", [T, HS, B, C], BF16, kind="ExternalInput")
    u0b_in = nc.dram_tensor("u0b", [B2, C, T, H], F32, kind="ExternalInput")
    trimask_in = nc.dram_tensor("trimask", [T, T], F32, kind="ExternalInput")
    ident_in = nc.dram_tensor("ident", [T, T], F32, kind="ExternalInput")
    par_in = {}
    for l in (0, 1):
        for nm in ("ldt", "lare", "aim", "cre", "cim"):
            par_in[(nm, l)] = nc.dram_tensor(f"{nm}{l}", [128, NHB], F32, kind="ExternalInput")
        par_in[("drep", l)] = nc.dram_tensor(f"drep{l}", [128, HS], F32, kind="ExternalInput")
        par_in[("wt", l)] = nc.dram_tensor(f"wt{l}", [H, 2 * H], BF16, kind="ExternalInput")
        par_in[("brow", l)] = nc.dram_tensor(f"brow{l}", [1, 2 * H], F32, kind="ExternalInput")

    HH = HS // 2   # h-half for split y collectives
    a2a_y_in = [[nc.dram_tensor(f"a2aY{l}{p}_in", [CORES, HH, B2, L], BF16) for p in (0, 1)]
                for l in (0, 1)]
    a2a_y_out = [[nc.dram_tensor(f"a2aY{l}{p}_out", [CORES, HH, B2, L], BF16) for p in (0, 1)]
                 for l in (0, 1)]
    a2a_u_in = [nc.dram_tensor(f"a2aU{b2}_in", [CORES, T, HS, C], BF16) for b2 in range(B2)]
    a2a_u_out = [nc.dram_tensor(f"a2aU{b2}_out", [CORES, T, HS, C], BF16) for b2 in range(B2)]
    out_z = nc.dram_tensor("out", [B2, L, H], F32, kind="ExternalOutput")

    with TileContext(nc) as tc, contextlib.ExitStack() as top:
        cpool = top.enter_context(tc.tile_pool(name="consts", bufs=1))
        trimask = cpool.tile([T, T], F32, tag="trimask", name="trimask")
        ident = cpool.tile([T, T], F32, tag="ident", name="ident")
        nc.sync.dma_start(trimask[:], trimask_in[:])
        nc.sync.dma_start(ident[:], ident_in[:])
        csts = cpool.tile([128, 32], F32, tag="csts", name="csts")
        SINC = [1.0, -1.0 / 6, 1.0 / 120, -1.0 / 5040, 1.0 / 362880, -1.0 / 39916800]
        COSC = [1.0, -1.0 / 2, 1.0 / 24, -1.0 / 720, 1.0 / 40320, -1.0 / 3628800]
        for k in range(6):
            nc.any.memset(csts[:, k : k + 1], SINC[k])
            nc.any.memset(csts[:, 6 + k : 7 + k], COSC[k])
        nc.any.memset(csts[:, 12:13], -1.0)
        nc.any.memset(csts[:, 13:14], 2.0)
        nc.any.memset(csts[:, 14:15], 1.0 / 16)
        import math
        for k in range(11):
            nc.any.memset(csts[:, 16 + k : 17 + k], 1.0 / math.factorial(k))
        nc.any.memset(csts[:, 27:28], 1.0 / 8)

        upool = top.enter_context(tc.tile_pool(name="u", bufs=1))
        u_sb = upool.tile([T, HS * B * C], BF16, tag="u_sb", name="u_sb")  # [j,(h,b,c)]
        nc.sync.dma_start(u_sb[:], u0_in[:].rearrange("j h b c -> j (h b c)"))

        def u_slice(h, bq=None):
            b0, nb = (0, B) if bq is None else (bq * 8, 8)
            ap = u_sb[:].rearrange("j (h b c) -> j h b c", b=B, c=C)
            return ap[:, h, b0 : b0 + nb, :]

        for l in (0, 1):
            # ======== CONV PHASE (H-shard) ========
            with contextlib.ExitStack() as cv:
                pp = cv.enter_context(tc.tile_pool(name=f"par{l}", bufs=1))
                P = {}
                for nm in ("ldt", "lare", "aim", "cre", "cim"):
                    P[nm] = pp.tile([128, NHB], F32, tag=f"p_{nm}", name=f"p_{nm}")
                    nc.sync.dma_start(P[nm][:], par_in[(nm, l)][:])
                drep = pp.tile([128, HS], F32, tag="p_drep", name="p_drep")
                nc.sync.dma_start(drep[:], par_in[("drep", l)][:])

                def wk(tag):
                    return pp.tile([128, NHB], F32, tag=tag, name=tag)[:]

                neg1 = csts[:, 12:13]
                two = csts[:, 13:14]
                s16 = csts[:, 14:15]

                def exp_poly(out, x):
                    """out = e^x via (T10(x/8))^8; |x| <= 8. Accurate to ~1e-7."""
                    ea = wk("exp_a")
                    et = wk("exp_t")
                    v.tensor_scalar(ea, x, csts[:, 27:28], None, op0=OP.mult)  # y = x/8
                    v.tensor_scalar(et, ea, csts[:, 26:27], csts[:, 25:26], op0=OP.mult, op1=OP.add)
                    for k in range(8, -1, -1):
                        v.tensor_mul(et, et, ea)
                        v.tensor_scalar(et, et, csts[:, 16 + k : 17 + k], None, op0=OP.add)
                    for _ in range(3):
                        v.tensor_mul(et, et, et)
                    v.tensor_copy(out, et)

                dt, eA = wk("dt"), wk("eA")
                exp_poly(dt, P["ldt"][:])
                exp_poly(eA, P["lare"][:])
                dtAre, dtAim = wk("dtAre"), wk("dtAim")
                v.scalar_tensor_tensor(dtAre, dt, -1.0, eA, op0=OP.mult, op1=OP.mult)
                v.tensor_mul(dtAim, dt, P["aim"][:])
                mag = wk("mag")
                exp_poly(mag, dtAre)
                q, x2 = wk("q"), wk("x2")
                v.tensor_scalar(q, dtAim, s16, None, op0=OP.mult)
                v.tensor_mul(x2, q, q)
                acc, t1, t2 = wk("acc"), wk("t1"), wk("t2")
                cr, ci = wk("cr"), wk("ci")
                v.tensor_scalar(acc, x2, csts[:, 5:6], csts[:, 4:5], op0=OP.mult, op1=OP.add)
                for k in (3, 2, 1, 0):
                    v.tensor_mul(t1, acc, x2)
                    v.tensor_scalar(acc, t1, csts[:, k : k + 1], None, op0=OP.add)
                v.tensor_mul(ci, acc, q)
                v.tensor_scalar(acc, x2, csts[:, 11:12], csts[:, 10:11], op0=OP.mult, op1=OP.add)
                for k in (9, 8, 7, 6):
                    v.tensor_mul(t1, acc, x2)
                    v.tensor_scalar(acc, t1, csts[:, k : k + 1], None, op0=OP.add)
                v.tensor_copy(cr, acc)
                for _ in range(4):
                    v.tensor_mul(t1, cr, cr)
                    v.tensor_mul(t2, ci, ci)
                    v.scalar_tensor_tensor(acc, cr, 2.0, ci, op0=OP.mult, op1=OP.mult)
                    v.tensor_sub(cr, t1, t2)
                    v.tensor_copy(ci, acc)
                wre, wim = wk("wre"), wk("wim")
                v.tensor_mul(wre, mag, cr)
                v.tensor_mul(wim, mag, ci)
                m2, im2 = wk("m2"), wk("im2")
                v.tensor_mul(m2, mag, mag)
                v.reciprocal(im2, m2)
                rpre, rpim = wk("rpre"), wk("rpim")
                v.tensor_mul(rpre, wre, im2)
                v.tensor_mul(rpim, wim, im2)
                wm1re = wk("wm1re")
                v.tensor_scalar(wm1re, wre, neg1, None, op0=OP.add)
                tre, tim = wk("tre"), wk("tim")
                v.tensor_mul(t1, P["cre"][:], wm1re)
                v.tensor_mul(t2, P["cim"][:], wim)
                v.tensor_sub(tre, t1, t2)
                v.tensor_mul(t1, P["cre"][:], wim)
                v.tensor_mul(t2, P["cim"][:], wm1re)
                v.tensor_add(tim, t1, t2)
                den, invd = wk("den"), wk("invd")
                v.tensor_mul(t1, eA, eA)
                v.tensor_mul(t2, P["aim"][:], P["aim"][:])
                v.tensor_add(den, t1, t2)
                v.reciprocal(invd, den)
                ccr, cci = wk("ccr"), wk("cci")
                v.tensor_mul(t1, tre, eA)
                v.tensor_mul(t2, tim, P["aim"][:])
                v.tensor_sub(acc, t2, t1)
                v.tensor_mul(ccr, acc, invd)
                v.tensor_mul(t1, tre, P["aim"][:])
                v.tensor_mul(t2, tim, eA)
                v.tensor_add(acc, t1, t2)
                v.tensor_mul(t1, acc, invd)
                v.tensor_scalar(cci, t1, neg1, None, op0=OP.mult)
                esr, esi = wk("esr"), wk("esi")
                v.tensor_mul(t1, ccr, wre)
                v.tensor_mul(t2, cci, wim)
                v.tensor_sub(acc, t1, t2)
                v.tensor_scalar(esr, acc, two, None, op0=OP.mult)
                v.tensor_mul(t1, ccr, wim)
                v.tensor_mul(t2, cci, wre)
                v.tensor_add(acc, t1, t2)
                v.tensor_scalar(esi, acc, two, None, op0=OP.mult)
                wtr, wti = wk("wtr"), wk("wti")
                v.tensor_copy(wtr, wre)
                v.tensor_copy(wti, wim)
                for _ in range(7):
                    v.tensor_mul(t1, wtr, wtr)
                    v.tensor_mul(t2, wti, wti)
                    v.scalar_tensor_tensor(acc, wtr, 2.0, wti, op0=OP.mult, op1=OP.mult)
                    v.tensor_sub(wtr, t1, t2)
                    v.tensor_copy(wti, acc)
                dre, dim_ = wk("dre"), wk("dim")
                v.tensor_copy(dre, wtr)
                v.tensor_scalar(dim_, wti, neg1, None, op0=OP.mult)

                # ---------- full-width tables ----------
                tpr = cv.enter_context(tc.tile_pool(name=f"tabR{l}", bufs=1))
                Rp_re = tpr.tile([128, NHB * T], F32, tag="Rp_re", name="Rp_re")
                Rp_im = tpr.tile([128, NHB * T], F32, tag="Rp_im", name="Rp_im")
                R16re = tpr.tile([128, NHB * T], BF16, tag="R16re", name="R16re")
                R16im = tpr.tile([128, NHB * T], BF16, tag="R16im", name="R16im")
                with tc.tile_pool(name=f"dblR{l}", bufs=1) as dwk:
                    _build_table(v, Rp_re, Rp_im, rpre[:], rpim[:],
                                 rpre[:], rpim[:], dwk, NHB)
                sc.activation(R16re[:], Rp_re[:], AF.Copy)
                v.tensor_copy(R16im[:], Rp_im[:])

                tpe = cv.enter_context(tc.tile_pool(name=f"tabE{l}", bufs=1))
                E_re = tpe.tile([128, NHB * T], F32, tag="E_re", name="E_re")
                E_im = tpe.tile([128, NHB * T], F32, tag="E_im", name="E_im")
                E16re = tpe.tile([128, NHB * T], BF16, tag="E16re", name="E16re")
                E16im = tpe.tile([128, NHB * T], BF16, tag="E16im", name="E16im")
                with tc.tile_pool(name=f"dblE{l}", bufs=1) as dwk:
                    _build_table(gp, E_re, E_im, esr[:], esi[:],
                                 wre[:], wim[:], dwk, NHB)
                sc.activation(E16re[:], E_re[:], AF.Copy)
                v.tensor_copy(E16im[:], E_im[:])

                # ---------- collection: X[p, (hb, b, c)] chunk states ----------
                stp = cv.enter_context(tc.tile_pool(name=f"st{l}", bufs=1))
                X_re = stp.tile([128, NHB * B * C], BF16, tag="X_re", name="X_re")
                X_im = stp.tile([128, NHB * B * C], BF16, tag="X_im", name="X_im")
                X_re4 = X_re[:].rearrange("p (h b c) -> p h b c", b=B, c=C)
                X_im4 = X_im[:].rearrange("p (h b c) -> p h b c", b=B, c=C)

                with tc.tile_pool(name=f"wsl{l}", bufs=3) as wslp, \
                     tc.tile_pool(name=f"pst{l}", bufs=2, space="PSUM") as pstp, \
                     tc.tile_pool(name=f"psc{l}", bufs=2, space="PSUM") as pscp:
                    for k in range(NHB):
                        wsl = [wslp.tile([128, T], BF16, tag=f"wsl{comp}", name=f"wsl{comp}")
                               for comp in (0, 1)]
                        for comp, Rt in enumerate((Rp_re, Rp_im)):
                            psT = pstp.tile([128, T], F32, tag="psT", name="psT")
                            te.transpose(psT[:], Rt[:, k * T : (k + 1) * T], ident[:])
                            sc.activation(wsl[comp][:], psT[:], AF.Copy)
                        psr = pscp.tile([128, B * C], F32, tag="psr", name="psr")
                        psi = pscp.tile([128, B * C], F32, tag="psi", name="psi")
                        for hp in (0, 1):
                            h = 2 * k + hp
                            us = u_slice(h)
                            te.matmul(psr[64 * hp : 64 * hp + 64, :],
                                      wsl[0][:, 64 * hp : 64 * hp + 64], us, start=True, stop=True)
                            te.matmul(psi[64 * hp : 64 * hp + 64, :],
                                      wsl[1][:, 64 * hp : 64 * hp + 64], us, start=True, stop=True)
                        sc.activation(X_re4[:, k, :, :], psr[:], AF.Copy)
                        sc.activation(X_im4[:, k, :, :], psi[:], AF.Copy)

                # ---------- scan (in place: X becomes S_before) ----------
                # All-vector chain (in-order, no cross-engine sync); state
                # ping-pongs between parities so the X snapshots (sc/gp) are
                # off the critical path. bf16 temporaries halve DVE time.
                with tc.tile_pool(name=f"scan{l}", bufs=1) as sp:
                    def stile(nm, dt_=F32):
                        return sp.tile([128, NHB * B], dt_, tag=nm, name=nm)[:].rearrange(
                            "p (h b) -> p h b", b=B)
                    Spp = [[stile("Sr0"), stile("Sr1")], [stile("Si0"), stile("Si1")]]
                    t_r3, t_i3 = stile("tm_r", BF16), stile("tm_i", BF16)
                    w1r, w2r = stile("w1r", BF16), stile("w2r", BF16)
                    w1i, w2i = stile("w1i", BF16), stile("w2i", BF16)
                    nc.any.memset(Spp[0][0], 0.0)
                    nc.any.memset(Spp[1][0], 0.0)
                    dreb = stile("dreb", BF16)
                    dimb = stile("dimb", BF16)
                    gp.tensor_copy(dreb, dre[:].unsqueeze(2).broadcast_to([128, NHB, B]))
                    gp.tensor_copy(dimb, dim_[:].unsqueeze(2).broadcast_to([128, NHB, B]))
                    for ccc in range(C):
                        pr, nx = ccc % 2, (ccc + 1) % 2
                        Sr3, Si3 = Spp[0][pr], Spp[1][pr]
                        Srn, Sin = Spp[0][nx], Spp[1][nx]
                        xr, xi = X_re4[:, :, :, ccc], X_im4[:, :, :, ccc]
                        v.tensor_add(t_r3, Sr3, xr)
                        v.tensor_add(t_i3, Si3, xi)
                        sc.activation(xr, Sr3, AF.Copy)
                        sc.activation(xi, Si3, AF.Copy)
                        v.tensor_mul(w1r, t_r3, dreb)
                        v.tensor_mul(w2r, t_i3, dimb)
                        v.tensor_sub(Srn, w1r, w2r)
                        v.tensor_mul(w1i, t_i3, dreb)
                        v.tensor_mul(w2i, t_r3, dimb)
                        v.tensor_add(Sin, w1i, w2i)

                # ---------- per-h conv ----------
                gt_pool = cv.enter_context(tc.tile_pool(name=f"gt{l}", bufs=3))
                ya_pool = cv.enter_context(tc.tile_pool(name=f"ya{l}", bufs=2))
                gp_ps = cv.enter_context(tc.tile_pool(name=f"gps{l}", bufs=2, space="PSUM"))
                cv_ps = cv.enter_context(tc.tile_pool(name=f"cvps{l}", bufs=2, space="PSUM"))
                HG = 8
                yg = [None, None]
                for h in range(HS):
                    hp, hb = h & 1, h >> 1
                    base = 64 * hp
                    er = E16re[base : base + 64, hb * T : (hb + 1) * T]
                    ei = E16im[base : base + 64, hb * T : (hb + 1) * T]
                    rr = R16re[base : base + 64, hb * T : (hb + 1) * T]
                    ri = R16im[base : base + 64, hb * T : (hb + 1) * T]
                    psG = gp_ps.tile([128, T], F32, tag="psG", name="psG")
                    te.matmul(psG[:], rr, er, start=True, stop=False)
                    te.matmul(psG[:], ri, ei, start=False, stop=True)
                    Gsb = gt_pool.tile([128, T], F32, tag="Gsb", name="Gsb")
                    GTt = gt_pool.tile([128, T], F32, tag="GTt", name="GTt")
                    GT = gt_pool.tile([128, T], BF16, tag="GT", name="GT")
                    sc.activation(Gsb[:], psG[:], AF.Copy)
                    gp.tensor_mul(GTt[:], Gsb[:], trimask[:])
                    v.scalar_tensor_tensor(GT[:], ident[:], drep[:, h : h + 1], GTt[:],
                                           op0=OP.mult, op1=OP.add)
                    if h % HG == 0:
                        yg = [ya_pool.tile([128, HG * T], BF16, tag=f"yg{qq}", name=f"yg{qq}")
                              for qq in (0, 1)]
                    for qq in (0, 1):
                        ps = cv_ps.tile([128, T], F32, tag="ps", name="ps")
                        lu = u_slice(h, qq)
                        te.matmul(ps[:], lu, GT[:], start=True, stop=False)
                        lr = X_re[base : base + 64,
                                  hb * B * C + qq * 128 : hb * B * C + qq * 128 + 128]
                        li = X_im[base : base + 64,
                                  hb * B * C + qq * 128 : hb * B * C + qq * 128 + 128]
                        te.matmul(ps[:], lr, er, start=False, stop=False)
                        te.matmul(ps[:], li, ei, start=False, stop=True)
                        sc.activation(yg[qq][:, (h % HG) * T : (h % HG + 1) * T], ps[:],
                                      AF.Gelu_apprx_tanh)
                    if h % HG == HG - 1:
                        hg0 = h - HG + 1
                        p = hg0 // HH
                        hg0p = hg0 % HH
                        for qq in (0, 1):
                            ygv = yg[qq][:].rearrange("bc (hh2 j) -> bc hh2 j", j=T)
                            for dd in range(4):
                                d = qq * 4 + dd
                                dst = a2a_y_in[l][p][d, hg0p : hg0p + HG, :, :].rearrange(
                                    "hh2 b2 (c j) -> (b2 c) hh2 j", j=T)
                                nc.sync.dma_start(dst, ygv[32 * dd : 32 * dd + 32, :, :])
                        if h == HH - 1 or h == HS - 1:
                            gp.collective_compute(
                                "AllToAll", OP.bypass, replica_groups=RG,
                                ins=[a2a_y_in[l][p][:].opt()],
                                outs=[a2a_y_out[l][p][:].opt()])

            # ======== GLU PHASE (B-shard) ========
            with contextlib.ExitStack() as gl:
                gpool = gl.enter_context(tc.tile_pool(name=f"glu{l}", bufs=1))
                wtiles = [gpool.tile([128, 2 * H], BF16, tag=f"wt{k}", name=f"wt{k}") for k in range(4)]
                ytiles = [gpool.tile([128, B2 * L], BF16, tag=f"yk{k}", name=f"yk{k}") for k in range(4)]
                for kt in range(4):
                    nc.sync.dma_start(wtiles[kt][:], par_in[("wt", l)][128 * kt : 128 * (kt + 1), :])
                    for si in (0, 1):
                        s = 2 * kt + si
                        for p in (0, 1):
                            nc.sync.dma_start(
                                ytiles[kt][64 * si + HH * p : 64 * si + HH * (p + 1), :],
                                a2a_y_out[l][p][s].rearrange("h b2 ll -> h (b2 ll)"))
                bbf = gpool.tile([1, 2 * H], F32, tag="bbf", name="bbf")
                nc.sync.dma_start(bbf[:], par_in[("brow", l)][:])
                bb16 = gpool.tile([1, 2 * H], BF16, tag="bb16", name="bb16")
                v.tensor_copy(bb16[:], bbf[:])
                ones1 = gpool.tile([1, T], BF16, tag="ones1", name="ones1")
                nc.any.memset(ones1[:], 1.0)
                if l == 0:
                    zstage = [gpool.tile([128, H * C], BF16, tag=f"zst{b2}", name=f"zst{b2}")
                              for b2 in range(B2)]
                    zs3 = [z[:].rearrange("p (h c) -> p h c", c=C) for z in zstage]
                zps = gl.enter_context(tc.tile_pool(name=f"zps{l}", bufs=2, space="PSUM"))
                zwp = gl.enter_context(tc.tile_pool(name=f"zw{l}", bufs=3))
                ubp = gl.enter_context(tc.tile_pool(name=f"ub{l}", bufs=2))
                for b2 in range(B2):
                    if l == 0:
                        ub3 = ubp.tile([128, C * H], F32, tag="ub3", name="ub3")[:].rearrange(
                            "p (c h) -> p c h", c=C)
                        nc.sync.dma_start(ub3, u0b_in[b2].rearrange("c j h -> j c h"))
                    for ct in range(C):
                        psZ1 = zps.tile([128, H], F32, tag="psZ1", name="psZ1")
                        psZ2 = zps.tile([128, H], F32, tag="psZ2", name="psZ2")
                        for kt in range(4):
                            yst = ytiles[kt][:, b2 * L + ct * T : b2 * L + (ct + 1) * T]
                            te.matmul(psZ1[:], yst, wtiles[kt][:, :H],
                                      start=(kt == 0), stop=False)
                            te.matmul(psZ2[:], yst, wtiles[kt][:, H:],
                                      start=(kt == 0), stop=False)
                        te.matmul(psZ1[:], ones1[:], bb16[:, :H], start=False, stop=True)
                        te.matmul(psZ2[:], ones1[:], bb16[:, H:], start=False, stop=True)
                        sg = zwp.tile([128, H], F32, tag="sg", name="sg")
                        sc.activation(sg[:], psZ2[:], AF.Sigmoid)
                        if l == 0:
                            zt = zwp.tile([128, H], F32, tag="zt", name="zt")
                            v.tensor_mul(zt[:], psZ1[:], sg[:])
                            v.tensor_add(zs3[b2][:, :, ct], zt[:], ub3[:, ct, :])
                        else:
                            osb = zwp.tile([128, H], F32, tag="osb", name="osb")
                            v.tensor_mul(osb[:], psZ1[:], sg[:])
                            nc.sync.dma_start(out_z[b2, ct * T : (ct + 1) * T, :], osb[:])
                    if l == 0:
                        for d in range(CORES):
                            dst = a2a_u_in[b2][d].rearrange("j hh c -> j (hh c)")
                            nc.sync.dma_start(
                                dst, zstage[b2][:, d * HS * C : (d + 1) * HS * C])
                        gp.collective_compute(
                            "AllToAll", OP.bypass, replica_groups=RG,
                            ins=[a2a_u_in[b2][:].opt()], outs=[a2a_u_out[b2][:].opt()])
                if l == 0:
                    uv = u_sb[:].rearrange("j (h b c) -> j h b c", b=B, c=C)
                    for s in range(CORES):
                        for b2 in range(B2):
                            nc.sync.dma_start(
                                uv[:, :, 2 * s + b2, :],
                                a2a_u_out[b2][s])
    nc.finalize()
    _NC_CACHE[key] = nc
    return nc


# ====================== host side ======================

def _prep_core_inputs(core, x, pars):
    hs = slice(HS * core, HS * (core + 1))
    ins = {}
    xs = x[:, :, hs]                                    # (B, L, 64)
    u0 = xs.reshape(B, C, T, HS).transpose(2, 3, 0, 1)  # (j, h, b, c)
    ins["u0"] = np.ascontiguousarray(u0)
    xb = x[B2 * core : B2 * (core + 1)]                 # (2, L, H)
    ins["u0b"] = np.ascontiguousarray(xb.reshape(B2, C, T, H))
    ins["trimask"] = np.triu(np.ones((T, T), np.float32))
    ins["ident"] = np.eye(T, dtype=np.float32)

    def scan_layout(a):
        if a.ndim == 1:
            a = np.broadcast_to(a[:, None], (HS, N))
        return np.ascontiguousarray(
            a.reshape(NHB, 2, N).transpose(1, 2, 0).reshape(128, NHB))

    for l in (0, 1):
        ins[f"ldt{l}"] = scan_layout(pars[f"ldt{l}"][hs])
        ins[f"lare{l}"] = scan_layout(pars[f"lAre{l}"][hs])
        ins[f"aim{l}"] = scan_layout(pars[f"Aim{l}"][hs])
        ins[f"cre{l}"] = scan_layout(pars[f"Cre{l}"][hs])
        ins[f"cim{l}"] = scan_layout(pars[f"Cim{l}"][hs])
        ins[f"drep{l}"] = np.ascontiguousarray(
            np.broadcast_to(pars[f"D{l}"][hs][None, :], (128, HS)))
        ins[f"wt{l}"] = np.ascontiguousarray(pars[f"W{l}"].T)
        ins[f"brow{l}"] = np.ascontiguousarray(pars[f"b{l}"][None, :])
    out = {k: vv.astype(np.float32) for k, vv in ins.items()}
    out["u0"] = ins["u0"].astype(ml_dtypes.bfloat16)
    for l in (0, 1):
        out[f"wt{l}"] = ins[f"wt{l}"].astype(ml_dtypes.bfloat16)
    return out


def run(x, pars, debug=False, trace=False):
    nc = build_kernel(debug=debug)
    in_maps = [_prep_core_inputs(c, x, pars) for c in range(CORES)]
    r = run_bass_kernel_spmd(nc, in_maps, core_ids=list(range(CORES)), trace=trace)
    outs = np.stack([r.results[c]["out"] for c in range(CORES)])  # (8, 2, L, H)
    full = outs.reshape(B, L, H)
    return full, r


def kernel(**inputs):
    x = np.asarray(inputs["x"], dtype=np.float32)
    pars = {k: np.asarray(vv, dtype=np.float32) for k, vv in inputs.items() if k != "x"}
    full, _ = run(x, pars)
    return full
